# revision 15
# baseline (speedup 1.0000x reference)
"""Trainium2 Bass kernel v2 for nn_DecoderBlock_Mamba.

Sharding: 8 cores = (batch b in 0..3) x (state-half sigma in {0,1}).
Scan uses a tiled partition layout: partition p = j*16 + i holds state
(s_lo + j) and channel-group offset i; 8 channel-groups g cover d = 16g + i.
This makes the B/C broadcasts group-invariant (built once) and both dbx / y
multiplies all-SBUF-bf16 (2x DVE fast path). U/DT are staged to DRAM and
replicated into the tiled layout by 8 DMAs per group (DMA engines are idle).

Self-contained: hardcodes all shapes; no sibling imports.
"""
import numpy as np

C = 64
DI = 128
DS = 16
DR = 4
B = 4
H = 64
W = 64
L = H * W
NS = 8            # states per core
NG = 8            # channel groups (of 16) per core
NCORES = 8
NCH = 8           # L chunks of 512
CH = 512
EPS = 1e-5

_cached = {}


def _build_program(sim=False, phases=3):
    import concourse.bass as bass
    import concourse.bacc as bacc
    import concourse.mybir as mybir
    import concourse.tile as tile

    dt = mybir.dt
    f32 = dt.float32
    bf16 = dt.bfloat16
    Act = mybir.ActivationFunctionType
    Alu = mybir.AluOpType
    Axis = mybir.AxisListType

    nc = bacc.Bacc(None, target_bir_lowering=False)

    def din(name, shape, dtype=f32):
        return nc.dram_tensor(name, shape, dtype, kind="ExternalInput")

    ximgs_d = din("ximgs", [C, 5 * L], bf16)
    cf32_d = din("cf32", [128, 32])
    cbf_d = din("cbf", [128, 2688], bf16)

    out_d = nc.dram_tensor("out_f", [C, L], f32, kind="ExternalOutput")

    groups = [[0, 1], [2, 3], [4, 5], [6, 7]]

    with tile.TileContext(nc) as tc:
        with (
            tc.tile_pool(name="dram", bufs=1, space="DRAM") as dpool,
            tc.tile_pool(name="const", bufs=1) as cpool,
            tc.tile_pool(name="big", bufs=1) as bpool,
            tc.tile_pool(name="sm", bufs=2) as spool,
            tc.tile_pool(name="ud", bufs=2) as udpool,
            tc.tile_pool(name="da", bufs=2) as dapool,
            tc.tile_pool(name="ps", bufs=4, space="PSUM") as ps,
            tc.tile_pool(name="psy", bufs=4, space="PSUM") as psy,
        ):
            # ---- constants (packed: 2 DMAs) ----
            cf = cpool.tile([128, 32], f32)
            cb = cpool.tile([128, 2688], bf16)
            nc.sync.dma_start(cf[:], cf32_d[:])
            nc.sync.dma_start(cb[:], cbf_d[:])
            bn_s = cf[0:C, 0:1]
            bn_b = cf[0:C, 1:2]
            ip_b0 = cf[:, 2:3]
            ip_b1 = cf[:, 3:4]
            cd_b = cf[:, 4:5]
            dt_b = cf[:, 5:6]
            Dp = cf[:, 6:7]
            a_vec = cf[:, 8:16]          # per-group a scale [128, 8]

            ident = cb[:, 0:128]
            cw = cb[0:C, 128:448]
            ip_lhsT = cb[0:C, 448:704]
            M_dt = cb[:, 704:832]
            cdiag = cb[:, 832:1344]      # 4 diag taps [128, 4*128]
            W_B = cb[:, 1344:1472]       # fused B broadcast [128,128]
            W_C = cb[:, 1472:1600]       # fused C broadcast
            Rg = cb[:, 1600:2624]        # 8 x [128,128] reduce mats
            op_lhsT = cb[:, 2624:2688]

            # ---- persistent activations ----
            SEQ = bpool.tile([C, L], bf16)            # BN+ReLU out (residual)
            HN = bpool.tile([C, L], bf16)             # LN-normalized
            XM0 = bpool.tile([DI, L + 4], bf16, name="XM0", tag="YP5")       # conv1d in, data @ col 4
            ZS = bpool.tile([DI, L], bf16)            # silu(z)
            XC = bpool.tile([DI, L], bf16)
            ESB = bpool.tile([DI, L], bf16, tag="ESB")
            DT = bpool.tile([DI, L], bf16, name="DT", tag="YSUM")
            U = bpool.tile([DI, L], bf16, name="U", tag="ESB")
            BT = bpool.tile([DI, L], bf16, name="BT")      # B_tile (j slow)
            CT = bpool.tile([DI, L], bf16, name="CT")      # C_tile
            YPs = [bpool.tile([DI, L], bf16, name=f"YP{g}", tag=f"YP{g}")
                   for g in range(NG)]
            YSUM = bpool.tile([DI, L], bf16, name="YSUM", tag="YSUM")

            # staging DRAM for U/DT tiled reads
            ud_dram = dpool.tile([DI, 2 * L], bf16, tag="uddram")
            y_in_t = dpool.tile([4, DI, L // 4], bf16, tag="yin")
            y_out_t = dpool.tile([4, DI, L // 4], bf16, tag="yout")

            # Prime ACT's vector clock on the const DMAs
            warm = cpool.tile([128, 1], f32, tag="warm")
            nc.scalar.activation(warm[:], cf[:, 0:1], Act.Copy)
            warm2 = cpool.tile([128, 1], bf16, tag="warm2")
            nc.scalar.activation(warm2[:], cb[:, 0:1], Act.Copy)
            nc.vector.tensor_scalar_mul(XM0[:, 0:4], cf[:, 0:4], 0.0)

            IMGS = [bpool.tile([C, L], bf16, name=f"img{t}", tag=f"YP{t}")
                    for t in range(5)]
            for t in range(5):
                nc.sync.dma_start(IMGS[t][:], ximgs_d[:, t * L:(t + 1) * L])

            # ---- front conv: 5 accumulating taps + BN + ReLU ----
            for chi in range(NCH):
                sl = slice(chi * CH, (chi + 1) * CH)
                pc = ps.tile([C, CH], f32, tag="mm")
                for tap in range(5):
                    nc.tensor.matmul(pc[:], cw[:, tap * C:(tap + 1) * C],
                                     IMGS[tap][:, sl],
                                     start=(tap == 0), stop=(tap == 4))
                nc.scalar.activation(SEQ[:, sl], pc[:],
                                     Act.Relu, bias=bn_b, scale=bn_s)

            # ---- LayerNorm over channels, batched 4 blocks per op ----
            HN0 = bpool.tile([128, L // 2], bf16, name="HN0", tag="HN0")
            VARS = spool.tile([128, 32], f32, tag="VARS")
            SQV = spool.tile([128, 32], f32, tag="SQV")
            RSTD = spool.tile([128, 32], f32, tag="RSTD")
            for g in range(NCH):
                tps4 = ps.tile([128, 4, C], bf16, tag="mm")
                for k in range(4):
                    blk = g * 4 + k
                    nc.tensor.transpose(tps4[:, k, :],
                                        SEQ[:, blk * 128:(blk + 1) * 128],
                                        ident[0:C, 0:C])
                mu4 = spool.tile([128, 4], f32, tag="mu4")
                nc.vector.tensor_reduce(mu4[:], tps4[:], Axis.X, Alu.add)
                mun4 = spool.tile([128, 4], f32, tag="mun4")
                nc.vector.tensor_scalar_mul(mun4[:], mu4[:], 1.0 / C)
                h04 = HN0[:, g * 256:(g + 1) * 256].rearrange(
                    "p (b c) -> p b c", b=4)
                nc.vector.tensor_tensor(h04, tps4[:],
                                        mun4[:].to_broadcast((128, 4, C)),
                                        op=Alu.subtract)
                sq4 = spool.tile([128, 4, C], f32, tag="sq4")
                nc.gpsimd.tensor_tensor(sq4[:], h04, h04, op=Alu.mult)
                ssq4 = spool.tile([128, 4], f32, tag="ssq4")
                nc.vector.tensor_reduce(ssq4[:], sq4[:], Axis.X, Alu.add)
                nc.vector.tensor_scalar(VARS[:, g * 4:(g + 1) * 4], ssq4[:],
                                        1.0 / C, EPS,
                                        op0=Alu.mult, op1=Alu.add)
                nc.scalar.activation(SQV[:, g * 4:(g + 1) * 4],
                                     VARS[:, g * 4:(g + 1) * 4], Act.Sqrt)
                nc.vector.reciprocal(RSTD[:, g * 4:(g + 1) * 4],
                                     SQV[:, g * 4:(g + 1) * 4])
            HNT = bpool.tile([128, L // 2], bf16, name="HNT", tag="HNT")
            for g in range(NCH):
                hnT4 = HNT[:, g * 256:(g + 1) * 256].rearrange(
                    "p (b c) -> p b c", b=4)
                nc.gpsimd.tensor_tensor(
                    hnT4, HN0[:, g * 256:(g + 1) * 256].rearrange(
                        "p (b c) -> p b c", b=4),
                    RSTD[:, g * 4:(g + 1) * 4].to_broadcast((128, 4, C)),
                    op=Alu.mult)
                tb4 = ps.tile([C, 4, 128], bf16, tag="mm")
                for k in range(4):
                    blk = g * 4 + k
                    nc.tensor.transpose(tb4[:, k, :],
                                        HNT[:, blk * C:(blk + 1) * C],
                                        ident)
                nc.vector.tensor_scalar(
                    HN[:, g * CH:(g + 1) * CH],
                    tb4[:].rearrange("p a b -> p (a b)"), 0.0,
                    None, op0=Alu.add)

            # ---- in_proj: xm (DVE bias-add) + z (ACT silu) ----
            for chi in range(NCH):
                sl = slice(chi * CH, (chi + 1) * CH)
                xm_ps = ps.tile([DI, CH], f32, tag="mm")
                z_ps = ps.tile([DI, CH], f32, tag="mm")
                nc.tensor.matmul(xm_ps[:], ip_lhsT[0:C, 0:DI], HN[:, sl],
                                 start=True, stop=True)
                nc.tensor.matmul(z_ps[:], ip_lhsT[0:C, DI:2 * DI], HN[:, sl],
                                 start=True, stop=True)
                nc.vector.tensor_scalar(XM0[:, 4 + chi * CH:4 + (chi + 1) * CH],
                                        xm_ps[:], ip_b0, None, op0=Alu.add)
                nc.scalar.activation(ZS[:, sl], z_ps[:], Act.Silu, bias=ip_b1)

            # ---- causal conv1d on PE (4 diag taps) + silu ----
            for chi in range(NCH):
                sl = slice(chi * CH, (chi + 1) * CH)
                cc = ps.tile([DI, CH], f32, tag="mm")
                for tap in range(4):
                    nc.tensor.matmul(cc[:], cdiag[:, tap * 128:(tap + 1) * 128],
                                     XM0[:, 1 + tap + chi * CH:
                                         1 + tap + chi * CH + CH],
                                     start=(tap == 0), stop=(tap == 3))
                nc.scalar.activation(XC[:, sl], cc[:], Act.Silu, bias=cd_b)

            # ---- x_proj: fused dt matmul; B/C rows; esb exp ----
            for chi in range(NCH):
                sl = slice(chi * CH, (chi + 1) * CH)
                dt_ps = ps.tile([DI, CH], f32, tag="mm")
                nc.tensor.matmul(dt_ps[:], M_dt, XC[:, sl],
                                 start=True, stop=True)
                nc.scalar.activation(ESB[:, sl], dt_ps[:], Act.Exp, bias=dt_b)


            # ---- DT = ln(1+esb) in halves (costs ~2 extra act-table loads
            # but lets U/staging start at the front's midpoint);
            # U = DT*XC; stage U/DT to DRAM for tiled replication ----
            LHf = L // 2
            for hf in range(2):
                hsl = slice(hf * LHf, (hf + 1) * LHf)
                nc.scalar.activation(DT[:, hsl], ESB[:, hsl], Act.Ln, bias=1.0)
                nc.vector.tensor_mul(U[:, hsl], DT[:, hsl], XC[:, hsl])
                nc.sync.dma_start(ud_dram[:, hf * LHf:(hf + 1) * LHf],
                                  U[:, hsl])
                nc.sync.dma_start(ud_dram[:, L + hf * LHf:L + (hf + 1) * LHf],
                                  DT[:, hsl])

            # ---- B_tile / C_tile (group-invariant): sel matmul + copy ----
            for chi in range(NCH):
                sl = slice(chi * CH, (chi + 1) * CH)
                bt_ps = ps.tile([DI, CH], f32, tag="mm")
                nc.tensor.matmul(bt_ps[:], W_B, XC[:, sl],
                                 start=True, stop=True)
                nc.scalar.activation(BT[:, sl], bt_ps[:], Act.Copy)
                ct_ps = ps.tile([DI, CH], f32, tag="mm")
                nc.tensor.matmul(ct_ps[:], W_C, XC[:, sl],
                                 start=True, stop=True)
                nc.vector.tensor_scalar(CT[:, sl], ct_ps[:], 0.0, None,
                                        op0=Alu.add)

            # ---- XCD = XC*Dp on Pool (runs during scan phase) ----
            XCD = bpool.tile([DI, L], bf16, name="XCD", tag="ESB")
            for hf in range(2):
                hsl = slice(hf * (L // 2), (hf + 1) * (L // 2))
                nc.gpsimd.tensor_tensor(XCD[:, hsl], XC[:, hsl],
                                        Dp.to_broadcast((DI, L // 2)),
                                        op=Alu.mult)

            # ---- scan phase: per channel-group g ----
            LH0 = L // 2
            ypsA = [psy.tile([DI, CH], f32, name=f"ypsA{ci}", tag="yps")
                    for ci in range(4)]
            for g in range(NG):
                udt = udpool.tile([DI, 2 * L], bf16, tag="udt")
                for j in range(NS):
                    nc.sync.dma_start(
                        udt[j * 16:(j + 1) * 16, :],
                        ud_dram[g * 16:(g + 1) * 16, :])
                for hf in range(2):
                    hsl = slice(hf * LH0, (hf + 1) * LH0)
                    dA = dapool.tile([DI, LH0], f32, tag="dA")
                    nc.scalar.activation(dA[:],
                                         udt[:, L + hf * LH0:L + (hf + 1) * LH0],
                                         Act.Exp, scale=a_vec[:, g:g + 1])
                    # dbx in-place into the U-half of udt (dead after this)
                    nc.vector.tensor_tensor(udt[:, hsl], udt[:, hsl],
                                            BT[:, hsl], op=Alu.mult)
                    init = 0.0 if hf == 0 else YPs[g][:, LH0 - 1:LH0]
                    nc.vector.tensor_tensor_scan(YPs[g][:, hsl], dA[:],
                                                 udt[:, hsl],
                                                 init, op0=Alu.mult, op1=Alu.add)
                # y partial: YP = H * C_tile (even groups on Pool)
                if g % 2 == 0:
                    nc.gpsimd.tensor_tensor(YPs[g][:], YPs[g][:], CT[:],
                                            op=Alu.mult)
                else:
                    nc.vector.tensor_tensor(YPs[g][:], YPs[g][:], CT[:],
                                            op=Alu.mult)
                # incremental y-reduce for chunks 0-3 (PSUM live across phase)
                for ci in range(4):
                    slc = slice(ci * CH, (ci + 1) * CH)
                    nc.tensor.matmul(ypsA[ci][:], Rg[:, g * 128:(g + 1) * 128],
                                     YPs[g][:, slc],
                                     start=(g == 0), stop=(g == NG - 1))

            # ---- y reduce: chunks 0-3 done incrementally; drain + chunks 4-7
            for chi in range(4):
                sl = slice(chi * CH, (chi + 1) * CH)
                ysb = spool.tile([DI, CH], bf16, tag="ysb")
                nc.scalar.activation(ysb[:], ypsA[chi][:], Act.Copy)
                nc.sync.dma_start(
                    y_in_t[chi // 2, :, (chi % 2) * CH:(chi % 2 + 1) * CH],
                    ysb[:])
            for chi in range(4, NCH):
                sl = slice(chi * CH, (chi + 1) * CH)
                yps = psy.tile([DI, CH], f32, tag="yps")
                for g in range(NG):
                    nc.tensor.matmul(yps[:], Rg[:, g * 128:(g + 1) * 128],
                                     YPs[g][:, sl],
                                     start=(g == 0), stop=(g == NG - 1))
                ysb = spool.tile([DI, CH], bf16, tag="ysb")
                if chi % 2 == 0:
                    nc.scalar.activation(ysb[:], yps[:], Act.Copy)
                else:
                    nc.vector.tensor_scalar(ysb[:], yps[:], 0.0, None,
                                            op0=Alu.add)
                nc.sync.dma_start(
                    y_in_t[chi // 2, :, (chi % 2) * CH:(chi % 2 + 1) * CH],
                    ysb[:])

            # ---- AllReduce partial y in quarters, pipelined with post/out ----
            # XCD precomputed on Pool (overlaps scan phase)
            YS = bpool.tile([DI, L], bf16, name="YS", tag="HN")
            LQ = L // 4
            for q in range(4):
                qsl = slice(q * LQ, (q + 1) * LQ)
                if sim:
                    nc.sync.dma_start(y_out_t[q], y_in_t[q])
                else:
                    nc.gpsimd.collective_compute(
                        "AllReduce", Alu.add, replica_groups=groups,
                        ins=[y_in_t[q].opt()], outs=[y_out_t[q].opt()])
                nc.sync.dma_start(YSUM[:, qsl], y_out_t[q])
                nc.vector.tensor_add(XCD[:, qsl], YSUM[:, qsl], XCD[:, qsl])
                nc.vector.tensor_mul(YS[:, qsl], XCD[:, qsl], ZS[:, qsl])
                for ci in range(2):
                    chi = q * 2 + ci
                    sl = slice(chi * CH, (chi + 1) * CH)
                    op_ps = psy.tile([C, CH], f32, tag="yps")
                    nc.tensor.matmul(op_ps[:], op_lhsT, YS[:, sl],
                                     start=True, stop=False)
                    nc.tensor.matmul(op_ps[:], ident[0:C, 0:C], SEQ[:, sl],
                                     start=False, stop=True)
                    outc = spool.tile([C, CH], f32, tag="outc")
                    if ci == 0:
                        nc.vector.tensor_scalar(outc[:], op_ps[:], 0.0, None,
                                                op0=Alu.add)
                    else:
                        nc.scalar.activation(outc[:], op_ps[:], Act.Copy)
                    nc.sync.dma_start(out_d[:, sl], outc[:])

    nc.compile()
    return nc


def _host_precompute(inp):
    import ml_dtypes
    f = lambda k: np.asarray(inp[k], np.float32)
    bf = lambda a: np.ascontiguousarray(a.astype(ml_dtypes.bfloat16))
    w1 = f("conv_w")[:, :, 0, 0]
    wh = f("dwh_w")[:, 0, :, 0]
    ww = f("dww_w")[:, 0, 0, :]
    taps = [
        w1 * (1.0 + wh[:, 1] + ww[:, 1])[None, :],   # center
        w1 * wh[:, 0][None, :],                       # up
        w1 * wh[:, 2][None, :],                       # down
        w1 * ww[:, 0][None, :],                       # left
        w1 * ww[:, 2][None, :],                       # right
    ]
    cw = np.concatenate([t.T for t in taps], axis=1)  # [cin=64, 5*64]
    btot = f("conv_b") + w1 @ (f("dwh_b") + f("dww_b"))
    s_bn = f("bn_g") / np.sqrt(f("bn_v") + EPS)
    bn_bias = s_bn * (btot - f("bn_m")) + f("bn_b")
    ipw = f("in_proj_w")
    ip_lhsT = (ipw * f("ln_g")[None, :]).T            # [64, 256]
    ip_bias = ipw @ f("ln_b")                          # [256]
    xpw = f("x_proj_w")                                # [36, 128]
    M_dt = f("dt_proj_w") @ xpw[:DR]                   # [128, 128]
    a_full = -np.exp(np.asarray(inp["A_log"], np.float32))  # [DI, DS]
    cdw = f("convd_w")[:, 0, :]                        # [128, 4]

    per_sigma = []
    for sg in range(2):
        s_lo = sg * NS
        cf32 = np.zeros((128, 32), np.float32)
        cf32[:C, 0] = s_bn
        cf32[:C, 1] = bn_bias
        cf32[:, 2] = ip_bias[:DI]
        cf32[:, 3] = ip_bias[DI:]
        cf32[:, 4] = f("convd_b")
        cf32[:, 5] = f("dt_proj_b")
        cf32[:, 6] = f("Dp")
        # a_vec per group g: a[p] = a_full[16g + p%16, s_lo + p//16]
        p = np.arange(128)
        for g in range(NG):
            cf32[:, 8 + g] = a_full[16 * g + p % 16, s_lo + p // 16]

        cbf = np.zeros((128, 2688), np.float32)
        cbf[:, 0:128] = np.eye(128, dtype=np.float32)
        cbf[:C, 128:448] = cw
        cbf[:C, 448:704] = ip_lhsT
        cbf[:, 704:832] = M_dt.T
        for tap in range(4):
            cbf[:, 832 + tap * 128:832 + (tap + 1) * 128] = np.diag(cdw[:, tap])
        # fused B/C broadcast: W_B[p, :] = xpw_B[s_lo + p//16, :] (stored T)
        for pp in range(128):
            cbf[:, 1344 + pp] = xpw[DR + s_lo + pp // 16]
            cbf[:, 1472 + pp] = xpw[DR + DS + s_lo + pp // 16]
        # Rg: R_g[p, d] = 1 iff d == 16g + p%16
        for g in range(NG):
            for pp in range(128):
                cbf[pp, 1600 + g * 128 + 16 * g + pp % 16] = 1.0
        cbf[:, 2624:2688] = f("out_proj_w").T
        per_sigma.append(dict(cf32=cf32, cbf=bf(cbf)))
    return {}, per_sigma


def _shift_images(xb):
    # 5 pre-shifted copies: ctr, up(reads h-1), dn(h+1), lf(w-1), rt(w+1)
    import ml_dtypes
    out = np.zeros((C, 5, H, W), np.float32)
    out[:, 0] = xb
    out[:, 1, 1:, :] = xb[:, :-1, :]
    out[:, 2, :-1, :] = xb[:, 1:, :]
    out[:, 3, :, 1:] = xb[:, :, :-1]
    out[:, 4, :, :-1] = xb[:, :, 1:]
    return np.ascontiguousarray(
        out.transpose(1, 0, 2, 3).reshape(5, C, L).transpose(1, 0, 2)
        .reshape(C, 5 * L).astype(ml_dtypes.bfloat16))


TRACE = False
LAST_EXEC_NS = None
LAST_TRACE_DIR = None


def kernel(**inputs):
    global LAST_EXEC_NS, LAST_TRACE_DIR
    from concourse.bass_utils import run_bass_kernel_spmd

    if "nc" not in _cached:
        _cached["nc"] = _build_program()
    nc = _cached["nc"]

    common, per_sigma = _host_precompute(inputs)
    x = np.asarray(inputs["x"], np.float32)
    in_maps = []
    for c in range(NCORES):
        b, sg = c // 2, c % 2
        m = dict(common)
        m.update(per_sigma[sg])
        m["ximgs"] = _shift_images(x[b])
        in_maps.append(m)

    kw = {}
    if TRACE:
        import tempfile
        LAST_TRACE_DIR = tempfile.mkdtemp(prefix="bass_trace_")
        kw = dict(trace=True, tmpdir=LAST_TRACE_DIR)
    r = run_bass_kernel_spmd(nc, in_maps, list(range(NCORES)), **kw)
    if r.exec_time_ns is not None:
        LAST_EXEC_NS = r.exec_time_ns
    res = r.results
    out = np.empty((B, C, H, W), np.float32)
    for b in range(B):
        out[b] = np.asarray(res[2 * b]["out_f"], np.float32).reshape(C, H, W)
    return out


# revision 28
# speedup vs baseline: 1.0089x; 1.0089x over previous
"""Trainium2 Bass kernel v2 for nn_DecoderBlock_Mamba.

Sharding: 8 cores = (batch b in 0..3) x (state-half sigma in {0,1}).
Scan uses a tiled partition layout: partition p = j*16 + i holds state
(s_lo + j) and channel-group offset i; 8 channel-groups g cover d = 16g + i.
This makes the B/C broadcasts group-invariant (built once) and both dbx / y
multiplies all-SBUF-bf16 (2x DVE fast path). U/DT are staged to DRAM and
replicated into the tiled layout by 8 DMAs per group (DMA engines are idle).

Self-contained: hardcodes all shapes; no sibling imports.
"""
import numpy as np

C = 64
DI = 128
DS = 16
DR = 4
B = 4
H = 64
W = 64
L = H * W
NS = 8            # states per core
NG = 8            # channel groups (of 16) per core
NCORES = 8
NCH = 8           # L chunks of 512
CH = 512
EPS = 1e-5

_cached = {}


def _build_program(sim=False, phases=3):
    import concourse.bass as bass
    import concourse.bacc as bacc
    import concourse.mybir as mybir
    import concourse.tile as tile

    dt = mybir.dt
    f32 = dt.float32
    bf16 = dt.bfloat16
    Act = mybir.ActivationFunctionType
    Alu = mybir.AluOpType
    Axis = mybir.AxisListType

    nc = bacc.Bacc(None, target_bir_lowering=False)

    def din(name, shape, dtype=f32):
        return nc.dram_tensor(name, shape, dtype, kind="ExternalInput")

    ximgs_d = din("ximgs", [C, 5 * L], bf16)
    cf32_d = din("cf32", [128, 32])
    cbf_d = din("cbf", [128, 2688], bf16)

    out_d = nc.dram_tensor("out_f", [C, L], f32, kind="ExternalOutput")

    groups = [[0, 1], [2, 3], [4, 5], [6, 7]]

    with tile.TileContext(nc) as tc:
        with (
            tc.tile_pool(name="dram", bufs=1, space="DRAM") as dpool,
            tc.tile_pool(name="const", bufs=1) as cpool,
            tc.tile_pool(name="big", bufs=1) as bpool,
            tc.tile_pool(name="sm", bufs=2) as spool,
            tc.tile_pool(name="ud", bufs=2) as udpool,
            tc.tile_pool(name="da", bufs=2) as dapool,
            tc.tile_pool(name="ps", bufs=4, space="PSUM") as ps,
            tc.tile_pool(name="psy", bufs=4, space="PSUM") as psy,
        ):
            # ---- constants (packed: 2 DMAs) ----
            cf = cpool.tile([128, 32], f32)
            cb = cpool.tile([128, 2688], bf16)
            nc.sync.dma_start(cf[:], cf32_d[:])
            nc.sync.dma_start(cb[:], cbf_d[:])
            bn_s = cf[0:C, 0:1]
            bn_b = cf[0:C, 1:2]
            ip_b0 = cf[:, 2:3]
            ip_b1 = cf[:, 3:4]
            cd_b = cf[:, 4:5]
            dt_b = cf[:, 5:6]
            Dp = cf[:, 6:7]
            a_vec = cf[:, 8:16]          # per-group a scale [128, 8]

            ident = cb[:, 0:128]
            cw = cb[0:C, 128:448]
            ip_lhsT = cb[0:C, 448:704]
            M_dt = cb[:, 704:832]
            cdiag = cb[:, 832:1344]      # 4 diag taps [128, 4*128]
            W_B = cb[:, 1344:1472]       # fused B broadcast [128,128]
            W_C = cb[:, 1472:1600]       # fused C broadcast
            Rg = cb[:, 1600:2624]        # 8 x [128,128] reduce mats
            op_lhsT = cb[:, 2624:2688]

            # ---- persistent activations ----
            SEQ = bpool.tile([C, L], bf16)            # BN+ReLU out (residual)
            HN = bpool.tile([C, L], bf16)             # LN-normalized
            XM0 = bpool.tile([DI, L + 4], bf16, name="XM0", tag="YP5")       # conv1d in, data @ col 4
            ZS = bpool.tile([DI, L], bf16)            # silu(z)
            XC = bpool.tile([DI, L], bf16)
            ESB = bpool.tile([DI, L], bf16, tag="ESB")
            DT = bpool.tile([DI, L], bf16, name="DT", tag="YSUM")
            U = bpool.tile([DI, L], bf16, name="U", tag="ESB")
            BT = bpool.tile([DI, L], bf16, name="BT")      # B_tile (j slow)
            CT = bpool.tile([DI, L], bf16, name="CT")      # C_tile
            YPs = [bpool.tile([DI, L], bf16, name=f"YP{g}", tag=f"YP{g}")
                   for g in range(NG)]
            YSUM = bpool.tile([DI, L], bf16, name="YSUM", tag="YSUM")

            # staging DRAM for U/DT tiled reads
            ud_dram = dpool.tile([DI, 2 * L], bf16, tag="uddram")
            y_in_t = dpool.tile([4, DI, L // 4], bf16, tag="yin")
            y_out_t = dpool.tile([4, DI, L // 4], bf16, tag="yout")

            # Prime ACT's vector clock on the const DMAs
            warm = cpool.tile([128, 1], f32, tag="warm")
            nc.scalar.activation(warm[:], cf[:, 0:1], Act.Copy)
            warm2 = cpool.tile([128, 1], bf16, tag="warm2")
            nc.scalar.activation(warm2[:], cb[:, 0:1], Act.Copy)
            nc.vector.tensor_scalar_mul(XM0[:, 0:4], cf[:, 0:4], 0.0)

            IMGS = [bpool.tile([C, L], bf16, name=f"img{t}", tag=f"YP{t}")
                    for t in range(5)]
            for t in range(5):
                nc.sync.dma_start(IMGS[t][:], ximgs_d[:, t * L:(t + 1) * L])

            # ---- front conv: 5 accumulating taps + BN + ReLU ----
            for chi in range(NCH):
                sl = slice(chi * CH, (chi + 1) * CH)
                pc = ps.tile([C, CH], f32, tag="mm")
                for tap in range(5):
                    nc.tensor.matmul(pc[:], cw[:, tap * C:(tap + 1) * C],
                                     IMGS[tap][:, sl],
                                     start=(tap == 0), stop=(tap == 4))
                nc.scalar.activation(SEQ[:, sl], pc[:],
                                     Act.Relu, bias=bn_b, scale=bn_s)

            # ---- LayerNorm over channels, batched 4 blocks per op ----
            HN0 = bpool.tile([128, L // 2], bf16, name="HN0", tag="HN0")
            VARS = spool.tile([128, 32], f32, tag="VARS")
            SQV = spool.tile([128, 32], f32, tag="SQV")
            RSTD = spool.tile([128, 32], f32, tag="RSTD")
            for g in range(NCH):
                tps4 = ps.tile([128, 4, C], bf16, tag="mm")
                for k in range(4):
                    blk = g * 4 + k
                    nc.tensor.transpose(tps4[:, k, :],
                                        SEQ[:, blk * 128:(blk + 1) * 128],
                                        ident[0:C, 0:C])
                mu4 = spool.tile([128, 4], f32, tag="mu4")
                nc.vector.tensor_reduce(mu4[:], tps4[:], Axis.X, Alu.add)
                mun4 = spool.tile([128, 4], f32, tag="mun4")
                nc.vector.tensor_scalar_mul(mun4[:], mu4[:], 1.0 / C)
                h04 = HN0[:, g * 256:(g + 1) * 256].rearrange(
                    "p (b c) -> p b c", b=4)
                nc.vector.tensor_tensor(h04, tps4[:],
                                        mun4[:].to_broadcast((128, 4, C)),
                                        op=Alu.subtract)
                sq4 = spool.tile([128, 4, C], f32, tag="sq4")
                nc.gpsimd.tensor_tensor(sq4[:], h04, h04, op=Alu.mult)
                ssq4 = spool.tile([128, 4], f32, tag="ssq4")
                nc.vector.tensor_reduce(ssq4[:], sq4[:], Axis.X, Alu.add)
                nc.vector.tensor_scalar(VARS[:, g * 4:(g + 1) * 4], ssq4[:],
                                        1.0 / C, EPS,
                                        op0=Alu.mult, op1=Alu.add)
                nc.scalar.activation(SQV[:, g * 4:(g + 1) * 4],
                                     VARS[:, g * 4:(g + 1) * 4], Act.Sqrt)
                nc.vector.reciprocal(RSTD[:, g * 4:(g + 1) * 4],
                                     SQV[:, g * 4:(g + 1) * 4])
            HNT = bpool.tile([128, L // 2], bf16, name="HNT", tag="HNT")
            for g in range(NCH):
                hnT4 = HNT[:, g * 256:(g + 1) * 256].rearrange(
                    "p (b c) -> p b c", b=4)
                nc.gpsimd.tensor_tensor(
                    hnT4, HN0[:, g * 256:(g + 1) * 256].rearrange(
                        "p (b c) -> p b c", b=4),
                    RSTD[:, g * 4:(g + 1) * 4].to_broadcast((128, 4, C)),
                    op=Alu.mult)
                tb4 = ps.tile([C, 4, 128], bf16, tag="mm")
                for k in range(4):
                    blk = g * 4 + k
                    nc.tensor.transpose(tb4[:, k, :],
                                        HNT[:, blk * C:(blk + 1) * C],
                                        ident)
                nc.vector.tensor_scalar(
                    HN[:, g * CH:(g + 1) * CH],
                    tb4[:].rearrange("p a b -> p (a b)"), 0.0,
                    None, op0=Alu.add)

            # ---- in_proj: xm (DVE bias-add) + z (ACT silu) ----
            for chi in range(NCH):
                sl = slice(chi * CH, (chi + 1) * CH)
                xm_ps = ps.tile([DI, CH], f32, tag="mm")
                z_ps = ps.tile([DI, CH], f32, tag="mm")
                nc.tensor.matmul(xm_ps[:], ip_lhsT[0:C, 0:DI], HN[:, sl],
                                 start=True, stop=True)
                nc.tensor.matmul(z_ps[:], ip_lhsT[0:C, DI:2 * DI], HN[:, sl],
                                 start=True, stop=True)
                nc.vector.tensor_scalar(XM0[:, 4 + chi * CH:4 + (chi + 1) * CH],
                                        xm_ps[:], ip_b0, None, op0=Alu.add)
                nc.scalar.activation(ZS[:, sl], z_ps[:], Act.Silu, bias=ip_b1)

            # ---- causal conv1d on PE (4 diag taps) + silu ----
            for chi in range(NCH):
                sl = slice(chi * CH, (chi + 1) * CH)
                cc = ps.tile([DI, CH], f32, tag="mm")
                for tap in range(4):
                    nc.tensor.matmul(cc[:], cdiag[:, tap * 128:(tap + 1) * 128],
                                     XM0[:, 1 + tap + chi * CH:
                                         1 + tap + chi * CH + CH],
                                     start=(tap == 0), stop=(tap == 3))
                nc.scalar.activation(XC[:, sl], cc[:], Act.Silu, bias=cd_b)

            # ---- x_proj: fused dt matmul; B/C rows; esb exp ----
            for chi in range(NCH):
                sl = slice(chi * CH, (chi + 1) * CH)
                dt_ps = ps.tile([DI, CH], f32, tag="mm")
                nc.tensor.matmul(dt_ps[:], M_dt, XC[:, sl],
                                 start=True, stop=True)
                nc.scalar.activation(ESB[:, sl], dt_ps[:], Act.Exp, bias=dt_b)


            # ---- DT = ln(1+esb) in halves (costs ~2 extra act-table loads
            # but lets U/staging start at the front's midpoint);
            # U = DT*XC; stage U/DT to DRAM for tiled replication ----
            LHf = L // 2
            for hf in range(2):
                hsl = slice(hf * LHf, (hf + 1) * LHf)
                nc.scalar.activation(DT[:, hsl], ESB[:, hsl], Act.Ln, bias=1.0)
                nc.vector.tensor_mul(U[:, hsl], DT[:, hsl], XC[:, hsl])
                nc.sync.dma_start(ud_dram[:, hf * LHf:(hf + 1) * LHf],
                                  U[:, hsl])
                nc.sync.dma_start(ud_dram[:, L + hf * LHf:L + (hf + 1) * LHf],
                                  DT[:, hsl])

            # ---- B_tile / C_tile (group-invariant): sel matmul + copy ----
            for chi in range(NCH):
                sl = slice(chi * CH, (chi + 1) * CH)
                bt_ps = ps.tile([DI, CH], f32, tag="mm")
                nc.tensor.matmul(bt_ps[:], W_B, XC[:, sl],
                                 start=True, stop=True)
                nc.scalar.activation(BT[:, sl], bt_ps[:], Act.Copy)
                ct_ps = ps.tile([DI, CH], f32, tag="mm")
                nc.tensor.matmul(ct_ps[:], W_C, XC[:, sl],
                                 start=True, stop=True)
                nc.vector.tensor_scalar(CT[:, sl], ct_ps[:], 0.0, None,
                                        op0=Alu.add)

            # ---- XCD = XC*Dp on Pool (runs during scan phase) ----
            XCD = bpool.tile([DI, L], bf16, name="XCD", tag="ESB")
            for hf in range(2):
                hsl = slice(hf * (L // 2), (hf + 1) * (L // 2))
                nc.gpsimd.tensor_tensor(XCD[:, hsl], XC[:, hsl],
                                        Dp.to_broadcast((DI, L // 2)),
                                        op=Alu.mult)

            # ---- scan phase: per channel-group g ----
            LH0 = L // 2
            ypsA = [psy.tile([DI, CH], f32, name=f"ypsA{ci}", tag="yps")
                    for ci in range(4)]
            for g in range(NG):
                udt = udpool.tile([DI, 2 * L], bf16, tag="udt")
                for j in range(NS):
                    nc.sync.dma_start(
                        udt[j * 16:(j + 1) * 16, :],
                        ud_dram[g * 16:(g + 1) * 16, :])
                for hf in range(2):
                    hsl = slice(hf * LH0, (hf + 1) * LH0)
                    dA = dapool.tile([DI, LH0], f32, tag="dA")
                    nc.scalar.activation(dA[:],
                                         udt[:, L + hf * LH0:L + (hf + 1) * LH0],
                                         Act.Exp, scale=a_vec[:, g:g + 1])
                    # dbx in-place into the U-half of udt (dead after this)
                    nc.vector.tensor_tensor(udt[:, hsl], udt[:, hsl],
                                            BT[:, hsl], op=Alu.mult)
                    init = 0.0 if hf == 0 else YPs[g][:, LH0 - 1:LH0]
                    nc.vector.tensor_tensor_scan(YPs[g][:, hsl], dA[:],
                                                 udt[:, hsl],
                                                 init, op0=Alu.mult, op1=Alu.add)
                # y partial: YP = H * C_tile (even groups on Pool)
                if g % 2 == 0:
                    nc.gpsimd.tensor_tensor(YPs[g][:], YPs[g][:], CT[:],
                                            op=Alu.mult)
                else:
                    nc.vector.tensor_tensor(YPs[g][:], YPs[g][:], CT[:],
                                            op=Alu.mult)
                # incremental y-reduce for chunks 0-3 (PSUM live across phase)
                for ci in range(4):
                    slc = slice(ci * CH, (ci + 1) * CH)
                    nc.tensor.matmul(ypsA[ci][:], Rg[:, g * 128:(g + 1) * 128],
                                     YPs[g][:, slc],
                                     start=(g == 0), stop=(g == NG - 1))

            # ---- y reduce: chunks 0-3 done incrementally; drain + chunks 4-7
            for chi in range(4):
                ysb = spool.tile([DI, CH], bf16, tag="ysb")
                nc.scalar.activation(ysb[:], ypsA[chi][:], Act.Copy)
                nc.sync.dma_start(
                    y_in_t[chi // 2, :, (chi % 2) * CH:(chi % 2 + 1) * CH],
                    ysb[:])
            for chi in range(4, NCH):
                sl = slice(chi * CH, (chi + 1) * CH)
                yps = psy.tile([DI, CH], f32, tag="yps")
                for g in range(NG):
                    nc.tensor.matmul(yps[:], Rg[:, g * 128:(g + 1) * 128],
                                     YPs[g][:, sl],
                                     start=(g == 0), stop=(g == NG - 1))
                ysb = spool.tile([DI, CH], bf16, tag="ysb")
                if chi % 2 == 0:
                    nc.scalar.activation(ysb[:], yps[:], Act.Copy)
                else:
                    nc.vector.tensor_scalar(ysb[:], yps[:], 0.0, None,
                                            op0=Alu.add)
                nc.sync.dma_start(
                    y_in_t[chi // 2, :, (chi % 2) * CH:(chi % 2 + 1) * CH],
                    ysb[:])

            # ---- AllReduce partial y in quarters, pipelined with post/out ----
            # XCD precomputed on Pool (overlaps scan phase)
            YS = bpool.tile([DI, L], bf16, name="YS", tag="HN")
            LQ = L // 4
            for q in range(4):
                qsl = slice(q * LQ, (q + 1) * LQ)
                if sim:
                    nc.sync.dma_start(y_out_t[q], y_in_t[q])
                else:
                    nc.gpsimd.collective_compute(
                        "AllReduce", Alu.add, replica_groups=groups,
                        ins=[y_in_t[q].opt()], outs=[y_out_t[q].opt()])
                nc.sync.dma_start(YSUM[:, qsl], y_out_t[q])
                nc.vector.tensor_add(XCD[:, qsl], YSUM[:, qsl], XCD[:, qsl])
                nc.vector.tensor_mul(YS[:, qsl], XCD[:, qsl], ZS[:, qsl])
                for ci in range(2):
                    chi = q * 2 + ci
                    sl = slice(chi * CH, (chi + 1) * CH)
                    op_ps = psy.tile([C, CH], f32, tag="yps")
                    nc.tensor.matmul(op_ps[:], op_lhsT, YS[:, sl],
                                     start=True, stop=True)
                    outc = spool.tile([C, CH], f32, tag="outc")
                    nc.vector.tensor_tensor(outc[:], op_ps[:], SEQ[:, sl],
                                            op=Alu.add)
                    nc.sync.dma_start(out_d[:, sl], outc[:])

    nc.compile()
    return nc


def _host_precompute(inp):
    import ml_dtypes
    f = lambda k: np.asarray(inp[k], np.float32)
    bf = lambda a: np.ascontiguousarray(a.astype(ml_dtypes.bfloat16))
    w1 = f("conv_w")[:, :, 0, 0]
    wh = f("dwh_w")[:, 0, :, 0]
    ww = f("dww_w")[:, 0, 0, :]
    taps = [
        w1 * (1.0 + wh[:, 1] + ww[:, 1])[None, :],   # center
        w1 * wh[:, 0][None, :],                       # up
        w1 * wh[:, 2][None, :],                       # down
        w1 * ww[:, 0][None, :],                       # left
        w1 * ww[:, 2][None, :],                       # right
    ]
    cw = np.concatenate([t.T for t in taps], axis=1)  # [cin=64, 5*64]
    btot = f("conv_b") + w1 @ (f("dwh_b") + f("dww_b"))
    s_bn = f("bn_g") / np.sqrt(f("bn_v") + EPS)
    bn_bias = s_bn * (btot - f("bn_m")) + f("bn_b")
    ipw = f("in_proj_w")
    ip_lhsT = (ipw * f("ln_g")[None, :]).T            # [64, 256]
    ip_bias = ipw @ f("ln_b")                          # [256]
    xpw = f("x_proj_w")                                # [36, 128]
    M_dt = f("dt_proj_w") @ xpw[:DR]                   # [128, 128]
    a_full = -np.exp(np.asarray(inp["A_log"], np.float32))  # [DI, DS]
    cdw = f("convd_w")[:, 0, :]                        # [128, 4]

    per_sigma = []
    for sg in range(2):
        s_lo = sg * NS
        cf32 = np.zeros((128, 32), np.float32)
        cf32[:C, 0] = s_bn
        cf32[:C, 1] = bn_bias
        cf32[:, 2] = ip_bias[:DI]
        cf32[:, 3] = ip_bias[DI:]
        cf32[:, 4] = f("convd_b")
        cf32[:, 5] = f("dt_proj_b")
        cf32[:, 6] = f("Dp")
        # a_vec per group g: a[p] = a_full[16g + p%16, s_lo + p//16]
        p = np.arange(128)
        for g in range(NG):
            cf32[:, 8 + g] = a_full[16 * g + p % 16, s_lo + p // 16]

        cbf = np.zeros((128, 2688), np.float32)
        cbf[:, 0:128] = np.eye(128, dtype=np.float32)
        cbf[:C, 128:448] = cw
        cbf[:C, 448:704] = ip_lhsT
        cbf[:, 704:832] = M_dt.T
        for tap in range(4):
            cbf[:, 832 + tap * 128:832 + (tap + 1) * 128] = np.diag(cdw[:, tap])
        # fused B/C broadcast: W_B[p, :] = xpw_B[s_lo + p//16, :] (stored T)
        for pp in range(128):
            cbf[:, 1344 + pp] = xpw[DR + s_lo + pp // 16]
            cbf[:, 1472 + pp] = xpw[DR + DS + s_lo + pp // 16]
        # Rg: R_g[p, d] = 1 iff d == 16g + p%16
        for g in range(NG):
            for pp in range(128):
                cbf[pp, 1600 + g * 128 + 16 * g + pp % 16] = 1.0
        cbf[:, 2624:2688] = f("out_proj_w").T
        per_sigma.append(dict(cf32=cf32, cbf=bf(cbf)))
    return {}, per_sigma


def _shift_images(xb):
    # 5 pre-shifted copies: ctr, up(reads h-1), dn(h+1), lf(w-1), rt(w+1)
    import ml_dtypes
    out = np.zeros((C, 5, H, W), np.float32)
    out[:, 0] = xb
    out[:, 1, 1:, :] = xb[:, :-1, :]
    out[:, 2, :-1, :] = xb[:, 1:, :]
    out[:, 3, :, 1:] = xb[:, :, :-1]
    out[:, 4, :, :-1] = xb[:, :, 1:]
    return np.ascontiguousarray(
        out.transpose(1, 0, 2, 3).reshape(5, C, L).transpose(1, 0, 2)
        .reshape(C, 5 * L).astype(ml_dtypes.bfloat16))


TRACE = False
LAST_EXEC_NS = None
LAST_TRACE_DIR = None


def kernel(**inputs):
    global LAST_EXEC_NS, LAST_TRACE_DIR
    from concourse.bass_utils import run_bass_kernel_spmd

    if "nc" not in _cached:
        _cached["nc"] = _build_program()
    nc = _cached["nc"]

    common, per_sigma = _host_precompute(inputs)
    x = np.asarray(inputs["x"], np.float32)
    in_maps = []
    for c in range(NCORES):
        b, sg = c // 2, c % 2
        m = dict(common)
        m.update(per_sigma[sg])
        m["ximgs"] = _shift_images(x[b])
        in_maps.append(m)

    kw = {}
    if TRACE:
        import tempfile
        LAST_TRACE_DIR = tempfile.mkdtemp(prefix="bass_trace_")
        kw = dict(trace=True, tmpdir=LAST_TRACE_DIR)
    r = run_bass_kernel_spmd(nc, in_maps, list(range(NCORES)), **kw)
    if r.exec_time_ns is not None:
        LAST_EXEC_NS = r.exec_time_ns
    res = r.results
    out = np.empty((B, C, H, W), np.float32)
    for b in range(B):
        out[b] = np.asarray(res[2 * b]["out_f"], np.float32).reshape(C, H, W)
    return out


# revision 35
# speedup vs baseline: 1.0266x; 1.0176x over previous
"""Trainium2 Bass kernel v2 for nn_DecoderBlock_Mamba.

Sharding: 8 cores = (batch b in 0..3) x (state-half sigma in {0,1}).
Scan uses a tiled partition layout: partition p = j*16 + i holds state
(s_lo + j) and channel-group offset i; 8 channel-groups g cover d = 16g + i.
This makes the B/C broadcasts group-invariant (built once) and both dbx / y
multiplies all-SBUF-bf16 (2x DVE fast path). U/DT are staged to DRAM and
replicated into the tiled layout by 8 DMAs per group (DMA engines are idle).

Self-contained: hardcodes all shapes; no sibling imports.
"""
import numpy as np

C = 64
DI = 128
DS = 16
DR = 4
B = 4
H = 64
W = 64
L = H * W
NS = 8            # states per core
NG = 8            # channel groups (of 16) per core
NCORES = 8
NCH = 8           # L chunks of 512
CH = 512
EPS = 1e-5

_cached = {}


def _build_program(sim=False, phases=3):
    import concourse.bass as bass
    import concourse.bacc as bacc
    import concourse.mybir as mybir
    import concourse.tile as tile

    dt = mybir.dt
    f32 = dt.float32
    bf16 = dt.bfloat16
    Act = mybir.ActivationFunctionType
    Alu = mybir.AluOpType
    Axis = mybir.AxisListType

    nc = bacc.Bacc(None, target_bir_lowering=False)

    def din(name, shape, dtype=f32):
        return nc.dram_tensor(name, shape, dtype, kind="ExternalInput")

    ximgs_d = din("ximgs", [C, 5 * L], bf16)
    cf32_d = din("cf32", [128, 32])
    cbf_d = din("cbf", [128, 2688], bf16)

    out_d = nc.dram_tensor("out_f", [C, L], f32, kind="ExternalOutput")

    groups = [[0, 1], [2, 3], [4, 5], [6, 7]]

    with tile.TileContext(nc) as tc:
        with (
            tc.tile_pool(name="dram", bufs=1, space="DRAM") as dpool,
            tc.tile_pool(name="const", bufs=1) as cpool,
            tc.tile_pool(name="big", bufs=1) as bpool,
            tc.tile_pool(name="sm", bufs=2) as spool,
            tc.tile_pool(name="ud", bufs=2) as udpool,
            tc.tile_pool(name="da", bufs=2) as dapool,
            tc.tile_pool(name="ps", bufs=4, space="PSUM") as ps,
            tc.tile_pool(name="psy", bufs=4, space="PSUM") as psy,
        ):
            # ---- constants (packed: 2 DMAs) ----
            cf = cpool.tile([128, 32], f32)
            cb = cpool.tile([128, 2688], bf16)
            nc.sync.dma_start(cf[:], cf32_d[:])
            nc.sync.dma_start(cb[:], cbf_d[:])
            bn_s = cf[0:C, 0:1]
            bn_b = cf[0:C, 1:2]
            ip_b0 = cf[:, 2:3]
            ip_b1 = cf[:, 3:4]
            cd_b = cf[:, 4:5]
            dt_b = cf[:, 5:6]
            Dp = cf[:, 6:7]
            a_vec = cf[:, 8:16]          # per-group a scale [128, 8]

            ident = cb[:, 0:128]
            cw = cb[0:C, 128:448]
            ip_lhsT = cb[0:C, 448:704]
            M_dt = cb[:, 704:832]
            cdiag = cb[:, 832:1344]      # 4 diag taps [128, 4*128]
            W_B = cb[:, 1344:1472]       # fused B broadcast [128,128]
            W_C = cb[:, 1472:1600]       # fused C broadcast
            Rg = cb[:, 1600:2624]        # 8 x [128,128] reduce mats
            op_lhsT = cb[:, 2624:2688]

            # ---- persistent activations ----
            SEQ = bpool.tile([C, L], bf16)            # BN+ReLU out (residual)
            HN = bpool.tile([C, L], bf16)             # LN-normalized
            XM0 = bpool.tile([DI, L + 4], bf16, name="XM0", tag="YP5")       # conv1d in, data @ col 4
            ZS = bpool.tile([DI, L], bf16)            # silu(z)
            XC = bpool.tile([DI, L], bf16)
            ESB = bpool.tile([DI, L], bf16, tag="ESB")
            DT = bpool.tile([DI, L], bf16, name="DT", tag="YSUM")
            U = bpool.tile([DI, L], bf16, name="U", tag="ESB")
            BT = bpool.tile([DI, L], bf16, name="BT")      # B_tile (j slow)
            CT = bpool.tile([DI, L], bf16, name="CT")      # C_tile
            YPs = [bpool.tile([DI, L], bf16, name=f"YP{g}", tag=f"YP{g}")
                   for g in range(NG)]
            YSUM = bpool.tile([DI, L], bf16, name="YSUM", tag="YSUM")

            # staging DRAM for U/DT tiled reads
            ud_dram = dpool.tile([DI, 2 * L], bf16, tag="uddram")
            y_in_t = dpool.tile([4, DI, L // 4], bf16, tag="yin")
            y_out_t = dpool.tile([4, DI, L // 4], bf16, tag="yout")

            # Prime ACT's vector clock on the const DMAs
            warm = cpool.tile([128, 1], f32, tag="warm")
            nc.scalar.activation(warm[:], cf[:, 0:1], Act.Copy)
            warm2 = cpool.tile([128, 1], bf16, tag="warm2")
            nc.scalar.activation(warm2[:], cb[:, 0:1], Act.Copy)
            nc.vector.tensor_scalar_mul(XM0[:, 0:4], cf[:, 0:4], 0.0)

            IMGS = [bpool.tile([C, L], bf16, name=f"img{t}", tag=f"YP{t}")
                    for t in range(5)]
            for t in range(5):
                nc.sync.dma_start(IMGS[t][:], ximgs_d[:, t * L:(t + 1) * L])

            # ---- front conv: 5 accumulating taps + BN + ReLU ----
            for chi in range(NCH):
                sl = slice(chi * CH, (chi + 1) * CH)
                pc = ps.tile([C, CH], f32, tag="mm")
                for tap in range(5):
                    nc.tensor.matmul(pc[:], cw[:, tap * C:(tap + 1) * C],
                                     IMGS[tap][:, sl],
                                     start=(tap == 0), stop=(tap == 4))
                nc.scalar.activation(SEQ[:, sl], pc[:],
                                     Act.Relu, bias=bn_b)

            # ---- LayerNorm over channels, batched 4 blocks per op ----
            HN0 = bpool.tile([128, L // 2], bf16, name="HN0", tag="HN0")
            VARS = spool.tile([128, 32], f32, tag="VARS")
            SQV = spool.tile([128, 32], f32, tag="SQV")
            RSTD = spool.tile([128, 32], f32, tag="RSTD")
            for g in range(NCH):
                tps4 = ps.tile([128, 4, C], bf16, tag="mm")
                for k in range(4):
                    blk = g * 4 + k
                    nc.tensor.transpose(tps4[:, k, :],
                                        SEQ[:, blk * 128:(blk + 1) * 128],
                                        ident[0:C, 0:C])
                mu4 = spool.tile([128, 4], f32, tag="mu4")
                nc.vector.tensor_reduce(mu4[:], tps4[:], Axis.X, Alu.add)
                mun4 = spool.tile([128, 4], f32, tag="mun4")
                nc.vector.tensor_scalar_mul(mun4[:], mu4[:], 1.0 / C)
                h04 = HN0[:, g * 256:(g + 1) * 256].rearrange(
                    "p (b c) -> p b c", b=4)
                nc.vector.tensor_tensor(h04, tps4[:],
                                        mun4[:].to_broadcast((128, 4, C)),
                                        op=Alu.subtract)
                sq4 = spool.tile([128, 4, C], f32, tag="sq4")
                nc.gpsimd.tensor_tensor(sq4[:], h04, h04, op=Alu.mult)
                ssq4 = spool.tile([128, 4], f32, tag="ssq4")
                nc.vector.tensor_reduce(ssq4[:], sq4[:], Axis.X, Alu.add)
                nc.vector.tensor_scalar(VARS[:, g * 4:(g + 1) * 4], ssq4[:],
                                        1.0 / C, EPS,
                                        op0=Alu.mult, op1=Alu.add)
                nc.scalar.activation(SQV[:, g * 4:(g + 1) * 4],
                                     VARS[:, g * 4:(g + 1) * 4], Act.Sqrt)
                nc.vector.reciprocal(RSTD[:, g * 4:(g + 1) * 4],
                                     SQV[:, g * 4:(g + 1) * 4])
            HNT = bpool.tile([128, L // 2], bf16, name="HNT", tag="HNT")
            for g in range(NCH):
                hnT4 = HNT[:, g * 256:(g + 1) * 256].rearrange(
                    "p (b c) -> p b c", b=4)
                nc.gpsimd.tensor_tensor(
                    hnT4, HN0[:, g * 256:(g + 1) * 256].rearrange(
                        "p (b c) -> p b c", b=4),
                    RSTD[:, g * 4:(g + 1) * 4].to_broadcast((128, 4, C)),
                    op=Alu.mult)
                tb4 = ps.tile([C, 4, 128], bf16, tag="mm")
                for k in range(4):
                    blk = g * 4 + k
                    nc.tensor.transpose(tb4[:, k, :],
                                        HNT[:, blk * C:(blk + 1) * C],
                                        ident)
                nc.vector.tensor_scalar(
                    HN[:, g * CH:(g + 1) * CH],
                    tb4[:].rearrange("p a b -> p (a b)"), 0.0,
                    None, op0=Alu.add)

            # ---- in_proj: xm (DVE bias-add) + z (ACT silu) ----
            for chi in range(NCH):
                sl = slice(chi * CH, (chi + 1) * CH)
                xm_ps = ps.tile([DI, CH], f32, tag="mm")
                z_ps = ps.tile([DI, CH], f32, tag="mm")
                nc.tensor.matmul(xm_ps[:], ip_lhsT[0:C, 0:DI], HN[:, sl],
                                 start=True, stop=True)
                nc.tensor.matmul(z_ps[:], ip_lhsT[0:C, DI:2 * DI], HN[:, sl],
                                 start=True, stop=True)
                nc.vector.tensor_scalar(XM0[:, 4 + chi * CH:4 + (chi + 1) * CH],
                                        xm_ps[:], ip_b0, None, op0=Alu.add)
                nc.scalar.activation(ZS[:, sl], z_ps[:], Act.Silu, bias=ip_b1)

            # ---- causal conv1d on PE (4 diag taps) + silu ----
            for chi in range(NCH):
                sl = slice(chi * CH, (chi + 1) * CH)
                cc = ps.tile([DI, CH], f32, tag="mm")
                for tap in range(4):
                    nc.tensor.matmul(cc[:], cdiag[:, tap * 128:(tap + 1) * 128],
                                     XM0[:, 1 + tap + chi * CH:
                                         1 + tap + chi * CH + CH],
                                     start=(tap == 0), stop=(tap == 3))
                nc.scalar.activation(XC[:, sl], cc[:], Act.Silu, bias=cd_b)

            # ---- x_proj: fused dt matmul; B/C rows; esb exp ----
            for chi in range(NCH):
                sl = slice(chi * CH, (chi + 1) * CH)
                dt_ps = ps.tile([DI, CH], f32, tag="mm")
                nc.tensor.matmul(dt_ps[:], M_dt, XC[:, sl],
                                 start=True, stop=True)
                nc.scalar.activation(ESB[:, sl], dt_ps[:], Act.Exp, bias=dt_b)


            # ---- DT = ln(1+esb) in halves (costs ~2 extra act-table loads
            # but lets U/staging start at the front's midpoint);
            # U = DT*XC; stage U/DT to DRAM for tiled replication ----
            LHf = L // 2
            for hf in range(2):
                hsl = slice(hf * LHf, (hf + 1) * LHf)
                nc.scalar.activation(DT[:, hsl], ESB[:, hsl], Act.Ln, bias=1.0)
                nc.vector.tensor_mul(U[:, hsl], DT[:, hsl], XC[:, hsl])
                nc.sync.dma_start(ud_dram[:, hf * LHf:(hf + 1) * LHf],
                                  U[:, hsl])
                nc.sync.dma_start(ud_dram[:, L + hf * LHf:L + (hf + 1) * LHf],
                                  DT[:, hsl])

            # ---- B_tile / C_tile (group-invariant): sel matmul + copy ----
            for chi in range(NCH):
                sl = slice(chi * CH, (chi + 1) * CH)
                bt_ps = ps.tile([DI, CH], f32, tag="mm")
                nc.tensor.matmul(bt_ps[:], W_B, XC[:, sl],
                                 start=True, stop=True)
                nc.scalar.activation(BT[:, sl], bt_ps[:], Act.Copy)
                ct_ps = ps.tile([DI, CH], f32, tag="mm")
                nc.tensor.matmul(ct_ps[:], W_C, XC[:, sl],
                                 start=True, stop=True)
                nc.vector.tensor_scalar(CT[:, sl], ct_ps[:], 0.0, None,
                                        op0=Alu.add)

            # ---- XCD = XC*Dp on Pool (runs during scan phase) ----
            XCD = bpool.tile([DI, L], bf16, name="XCD", tag="ESB")
            for hf in range(2):
                hsl = slice(hf * (L // 2), (hf + 1) * (L // 2))
                nc.gpsimd.tensor_tensor(XCD[:, hsl], XC[:, hsl],
                                        Dp.to_broadcast((DI, L // 2)),
                                        op=Alu.mult)

            # dummy exp: forces the exp act-table load to happen now (ACT
            # idle) instead of being chained onto the first dA exp's waits
            dumex = cpool.tile([128, 1], f32, tag="dumex")
            nc.scalar.activation(dumex[:], BT[:, 0:1], Act.Exp)

            # ---- scan phase: per channel-group g ----
            LH0 = L // 2
            ypsA = [psy.tile([DI, CH], f32, name=f"ypsA{ci}", tag="yps")
                    for ci in range(4)]
            for g in range(NG):
                udt = udpool.tile([DI, 2 * L], bf16, tag="udt")
                for j in range(NS):
                    nc.sync.dma_start(
                        udt[j * 16:(j + 1) * 16, :],
                        ud_dram[g * 16:(g + 1) * 16, :])
                for hf in range(2):
                    hsl = slice(hf * LH0, (hf + 1) * LH0)
                    dA = dapool.tile([DI, LH0], f32, tag="dA")
                    nc.scalar.activation(dA[:],
                                         udt[:, L + hf * LH0:L + (hf + 1) * LH0],
                                         Act.Exp, scale=a_vec[:, g:g + 1])
                    # dbx in-place into the U-half of udt (dead after this)
                    nc.vector.tensor_tensor(udt[:, hsl], udt[:, hsl],
                                            BT[:, hsl], op=Alu.mult)
                    init = 0.0 if hf == 0 else YPs[g][:, LH0 - 1:LH0]
                    nc.vector.tensor_tensor_scan(YPs[g][:, hsl], dA[:],
                                                 udt[:, hsl],
                                                 init, op0=Alu.mult, op1=Alu.add)
                # y partial: YP = H * C_tile (even groups on Pool), split in
                # halves so h0 overlaps the h1 scan and accumulates earlier
                for hf in range(2):
                    hsl2 = slice(hf * LH0, (hf + 1) * LH0)
                    if g % 2 == 0:
                        nc.gpsimd.tensor_tensor(YPs[g][:, hsl2], YPs[g][:, hsl2],
                                                CT[:, hsl2], op=Alu.mult)
                    else:
                        nc.vector.tensor_tensor(YPs[g][:, hsl2], YPs[g][:, hsl2],
                                                CT[:, hsl2], op=Alu.mult)
                # incremental y-reduce for chunks 0-3 (PSUM live across phase)
                for ci in range(4):
                    slc = slice(ci * CH, (ci + 1) * CH)
                    nc.tensor.matmul(ypsA[ci][:], Rg[:, g * 128:(g + 1) * 128],
                                     YPs[g][:, slc],
                                     start=(g == 0), stop=(g == NG - 1))

            # ---- y reduce: chunks 0-3 done incrementally; drain + chunks 4-7
            for chi in range(4):
                ysb = spool.tile([DI, CH], bf16, tag="ysb")
                nc.scalar.activation(ysb[:], ypsA[chi][:], Act.Copy)
                nc.sync.dma_start(
                    y_in_t[chi // 2, :, (chi % 2) * CH:(chi % 2 + 1) * CH],
                    ysb[:])
            for chi in range(4, NCH):
                sl = slice(chi * CH, (chi + 1) * CH)
                yps = psy.tile([DI, CH], f32, tag="yps")
                for g in range(NG):
                    nc.tensor.matmul(yps[:], Rg[:, g * 128:(g + 1) * 128],
                                     YPs[g][:, sl],
                                     start=(g == 0), stop=(g == NG - 1))
                ysb = spool.tile([DI, CH], bf16, tag="ysb")
                if chi % 2 == 0:
                    nc.scalar.activation(ysb[:], yps[:], Act.Copy)
                else:
                    nc.vector.tensor_scalar(ysb[:], yps[:], 0.0, None,
                                            op0=Alu.add)
                nc.sync.dma_start(
                    y_in_t[chi // 2, :, (chi % 2) * CH:(chi % 2 + 1) * CH],
                    ysb[:])

            # ---- AllReduce partial y in quarters, pipelined with post/out ----
            # XCD precomputed on Pool (overlaps scan phase)
            YS = bpool.tile([DI, L], bf16, name="YS", tag="HN")
            LQ = L // 4
            for q in range(4):
                qsl = slice(q * LQ, (q + 1) * LQ)
                if sim:
                    nc.sync.dma_start(y_out_t[q], y_in_t[q])
                else:
                    nc.gpsimd.collective_compute(
                        "AllReduce", Alu.add, replica_groups=groups,
                        ins=[y_in_t[q].opt()], outs=[y_out_t[q].opt()])
                nc.sync.dma_start(YSUM[:, qsl], y_out_t[q])
                nc.vector.tensor_add(XCD[:, qsl], YSUM[:, qsl], XCD[:, qsl])
                nc.vector.tensor_mul(YS[:, qsl], XCD[:, qsl], ZS[:, qsl])
                for ci in range(2):
                    chi = q * 2 + ci
                    sl = slice(chi * CH, (chi + 1) * CH)
                    op_ps = psy.tile([C, CH], f32, tag="yps")
                    nc.tensor.matmul(op_ps[:], op_lhsT, YS[:, sl],
                                     start=True, stop=True)
                    outc = spool.tile([C, CH], f32, tag="outc")
                    nc.vector.tensor_tensor(outc[:], op_ps[:], SEQ[:, sl],
                                            op=Alu.add)
                    nc.sync.dma_start(out_d[:, sl], outc[:])

    nc.compile()
    return nc


def _host_precompute(inp):
    import ml_dtypes
    f = lambda k: np.asarray(inp[k], np.float32)
    bf = lambda a: np.ascontiguousarray(a.astype(ml_dtypes.bfloat16))
    w1 = f("conv_w")[:, :, 0, 0]
    wh = f("dwh_w")[:, 0, :, 0]
    ww = f("dww_w")[:, 0, 0, :]
    s_bn = f("bn_g") / np.sqrt(f("bn_v") + EPS)
    taps = [
        w1 * (1.0 + wh[:, 1] + ww[:, 1])[None, :],   # center
        w1 * wh[:, 0][None, :],                       # up
        w1 * wh[:, 2][None, :],                       # down
        w1 * ww[:, 0][None, :],                       # left
        w1 * ww[:, 2][None, :],                       # right
    ]
    cw = np.concatenate([t.T for t in taps], axis=1)
    cw = cw * np.tile(s_bn, 5)[None, :]  # [cin=64, 5*64]
    btot = f("conv_b") + w1 @ (f("dwh_b") + f("dww_b"))
    bn_bias = s_bn * (btot - f("bn_m")) + f("bn_b")
    ipw = f("in_proj_w")
    ip_lhsT = (ipw * f("ln_g")[None, :]).T            # [64, 256]
    ip_bias = ipw @ f("ln_b")                          # [256]
    xpw = f("x_proj_w")                                # [36, 128]
    M_dt = f("dt_proj_w") @ xpw[:DR]                   # [128, 128]
    a_full = -np.exp(np.asarray(inp["A_log"], np.float32))  # [DI, DS]
    cdw = f("convd_w")[:, 0, :]                        # [128, 4]

    per_sigma = []
    for sg in range(2):
        s_lo = sg * NS
        cf32 = np.zeros((128, 32), np.float32)
        cf32[:C, 0] = s_bn
        cf32[:C, 1] = bn_bias
        cf32[:, 2] = ip_bias[:DI]
        cf32[:, 3] = ip_bias[DI:]
        cf32[:, 4] = f("convd_b")
        cf32[:, 5] = f("dt_proj_b")
        cf32[:, 6] = f("Dp")
        # a_vec per group g: a[p] = a_full[16g + p%16, s_lo + p//16]
        p = np.arange(128)
        for g in range(NG):
            cf32[:, 8 + g] = a_full[16 * g + p % 16, s_lo + p // 16]

        cbf = np.zeros((128, 2688), np.float32)
        cbf[:, 0:128] = np.eye(128, dtype=np.float32)
        cbf[:C, 128:448] = cw
        cbf[:C, 448:704] = ip_lhsT
        cbf[:, 704:832] = M_dt.T
        for tap in range(4):
            cbf[:, 832 + tap * 128:832 + (tap + 1) * 128] = np.diag(cdw[:, tap])
        # fused B/C broadcast: W_B[p, :] = xpw_B[s_lo + p//16, :] (stored T)
        for pp in range(128):
            cbf[:, 1344 + pp] = xpw[DR + s_lo + pp // 16]
            cbf[:, 1472 + pp] = xpw[DR + DS + s_lo + pp // 16]
        # Rg: R_g[p, d] = 1 iff d == 16g + p%16
        for g in range(NG):
            for pp in range(128):
                cbf[pp, 1600 + g * 128 + 16 * g + pp % 16] = 1.0
        cbf[:, 2624:2688] = f("out_proj_w").T
        per_sigma.append(dict(cf32=cf32, cbf=bf(cbf)))
    return {}, per_sigma


def _shift_images(xb):
    # 5 pre-shifted copies: ctr, up(reads h-1), dn(h+1), lf(w-1), rt(w+1)
    import ml_dtypes
    out = np.zeros((C, 5, H, W), np.float32)
    out[:, 0] = xb
    out[:, 1, 1:, :] = xb[:, :-1, :]
    out[:, 2, :-1, :] = xb[:, 1:, :]
    out[:, 3, :, 1:] = xb[:, :, :-1]
    out[:, 4, :, :-1] = xb[:, :, 1:]
    return np.ascontiguousarray(
        out.transpose(1, 0, 2, 3).reshape(5, C, L).transpose(1, 0, 2)
        .reshape(C, 5 * L).astype(ml_dtypes.bfloat16))


TRACE = False
LAST_EXEC_NS = None
LAST_TRACE_DIR = None


def kernel(**inputs):
    global LAST_EXEC_NS, LAST_TRACE_DIR
    from concourse.bass_utils import run_bass_kernel_spmd

    if "nc" not in _cached:
        _cached["nc"] = _build_program()
    nc = _cached["nc"]

    common, per_sigma = _host_precompute(inputs)
    x = np.asarray(inputs["x"], np.float32)
    in_maps = []
    for c in range(NCORES):
        b, sg = c // 2, c % 2
        m = dict(common)
        m.update(per_sigma[sg])
        m["ximgs"] = _shift_images(x[b])
        in_maps.append(m)

    kw = {}
    if TRACE:
        import tempfile
        LAST_TRACE_DIR = tempfile.mkdtemp(prefix="bass_trace_")
        kw = dict(trace=True, tmpdir=LAST_TRACE_DIR)
    r = run_bass_kernel_spmd(nc, in_maps, list(range(NCORES)), **kw)
    if r.exec_time_ns is not None:
        LAST_EXEC_NS = r.exec_time_ns
    res = r.results
    out = np.empty((B, C, H, W), np.float32)
    for b in range(B):
        out[b] = np.asarray(res[2 * b]["out_f"], np.float32).reshape(C, H, W)
    return out


# revision 41
# speedup vs baseline: 1.0374x; 1.0105x over previous
"""Trainium2 Bass kernel v2 for nn_DecoderBlock_Mamba.

Sharding: 8 cores = (batch b in 0..3) x (state-half sigma in {0,1}).
Scan uses a tiled partition layout: partition p = j*16 + i holds state
(s_lo + j) and channel-group offset i; 8 channel-groups g cover d = 16g + i.
This makes the B/C broadcasts group-invariant (built once) and both dbx / y
multiplies all-SBUF-bf16 (2x DVE fast path). U/DT are staged to DRAM and
replicated into the tiled layout by 8 DMAs per group (DMA engines are idle).

Self-contained: hardcodes all shapes; no sibling imports.
"""
import numpy as np

C = 64
DI = 128
DS = 16
DR = 4
B = 4
H = 64
W = 64
L = H * W
NS = 8            # states per core
NG = 8            # channel groups (of 16) per core
NCORES = 8
NCH = 8           # L chunks of 512
CH = 512
EPS = 1e-5

_cached = {}


def _build_program(sim=False, phases=3):
    import concourse.bass as bass
    import concourse.bacc as bacc
    import concourse.mybir as mybir
    import concourse.tile as tile

    dt = mybir.dt
    f32 = dt.float32
    bf16 = dt.bfloat16
    Act = mybir.ActivationFunctionType
    Alu = mybir.AluOpType
    Axis = mybir.AxisListType

    nc = bacc.Bacc(None, target_bir_lowering=False)

    def din(name, shape, dtype=f32):
        return nc.dram_tensor(name, shape, dtype, kind="ExternalInput")

    ximgs_d = din("ximgs", [C, 5 * L], bf16)
    cf32_d = din("cf32", [128, 32])
    cbf_d = din("cbf", [128, 2688], bf16)

    out_d = nc.dram_tensor("out_f", [C, L], f32, kind="ExternalOutput")

    groups = [[0, 1], [2, 3], [4, 5], [6, 7]]

    with tile.TileContext(nc) as tc:
        with (
            tc.tile_pool(name="dram", bufs=1, space="DRAM") as dpool,
            tc.tile_pool(name="const", bufs=1) as cpool,
            tc.tile_pool(name="big", bufs=1) as bpool,
            tc.tile_pool(name="sm", bufs=2) as spool,
            tc.tile_pool(name="ud", bufs=2) as udpool,
            tc.tile_pool(name="da", bufs=2) as dapool,
            tc.tile_pool(name="ps", bufs=4, space="PSUM") as ps,
            tc.tile_pool(name="psy", bufs=4, space="PSUM") as psy,
        ):
            # ---- constants (packed: 2 DMAs) ----
            cf = cpool.tile([128, 32], f32)
            cb = cpool.tile([128, 2688], bf16)
            nc.sync.dma_start(cf[:], cf32_d[:])
            nc.sync.dma_start(cb[:], cbf_d[:])
            bn_s = cf[0:C, 0:1]
            bn_b = cf[0:C, 1:2]
            ip_b0 = cf[:, 2:3]
            ip_b1 = cf[:, 3:4]
            cd_b = cf[:, 4:5]
            dt_b = cf[:, 5:6]
            Dp = cf[:, 6:7]
            a_vec = cf[:, 8:16]          # per-group a scale [128, 8]

            ident = cb[:, 0:128]
            cw = cb[0:C, 128:448]
            ip_lhsT = cb[0:C, 448:704]
            M_dt = cb[:, 704:832]
            cdiag = cb[:, 832:1344]      # 4 diag taps [128, 4*128]
            W_B = cb[:, 1344:1472]       # fused B broadcast [128,128]
            W_C = cb[:, 1472:1600]       # fused C broadcast
            Rg = cb[:, 1600:2624]        # 8 x [128,128] reduce mats
            op_lhsT = cb[:, 2624:2688]

            # ---- persistent activations ----
            SEQ = bpool.tile([C, L], bf16)            # BN+ReLU out (residual)
            HN = bpool.tile([C, L], bf16)             # LN-normalized
            XM0 = bpool.tile([DI, L + 4], bf16, name="XM0", tag="YP5")       # conv1d in, data @ col 4
            ZS = bpool.tile([DI, L], bf16)            # silu(z)
            XC = bpool.tile([DI, L], bf16)
            ESB = bpool.tile([DI, L], bf16, tag="ESB")
            DT = bpool.tile([DI, L], bf16, name="DT", tag="YSUM")
            U = bpool.tile([DI, L], bf16, name="U", tag="ESB")
            BT = bpool.tile([DI, L], bf16, name="BT")      # B_tile (j slow)
            CT = bpool.tile([DI, L], bf16, name="CT")      # C_tile
            YPs = [bpool.tile([DI, L], bf16, name=f"YP{g}", tag=f"YP{g}")
                   for g in range(NG)]
            YSUM = bpool.tile([DI, L], bf16, name="YSUM", tag="YSUM")

            # staging DRAM for U/DT tiled reads
            ud_dram = dpool.tile([DI, 2 * L], bf16, tag="uddram")
            y_in_t = dpool.tile([4, DI, L // 4], bf16, tag="yin")
            y_out_t = dpool.tile([4, DI, L // 4], bf16, tag="yout")

            # Prime ACT's vector clock on the const DMAs
            warm = cpool.tile([128, 1], f32, tag="warm")
            nc.scalar.activation(warm[:], cf[:, 0:1], Act.Copy)
            warm2 = cpool.tile([128, 1], bf16, tag="warm2")
            nc.scalar.activation(warm2[:], cb[:, 0:1], Act.Copy)
            nc.vector.tensor_scalar_mul(XM0[:, 0:4], cf[:, 0:4], 0.0)

            IMGS = [bpool.tile([C, L], bf16, name=f"img{t}", tag=f"YP{t}")
                    for t in range(5)]
            for t in range(5):
                nc.sync.dma_start(IMGS[t][:], ximgs_d[:, t * L:(t + 1) * L])

            # ---- front conv: 5 accumulating taps + BN + ReLU ----
            for chi in range(NCH):
                sl = slice(chi * CH, (chi + 1) * CH)
                pc = ps.tile([C, CH], f32, tag="mm")
                for tap in range(5):
                    nc.tensor.matmul(pc[:], cw[:, tap * C:(tap + 1) * C],
                                     IMGS[tap][:, sl],
                                     start=(tap == 0), stop=(tap == 4))
                nc.scalar.activation(SEQ[:, sl], pc[:],
                                     Act.Relu, bias=bn_b)

            # ---- LayerNorm over channels, batched 4 blocks per op ----
            HN0 = bpool.tile([128, L // 2], bf16, name="HN0", tag="HN0")
            VARS = spool.tile([128, 32], f32, tag="VARS")
            SQV = spool.tile([128, 32], f32, tag="SQV")
            RSTD = spool.tile([128, 32], f32, tag="RSTD")
            for g in range(NCH):
                tps4 = ps.tile([128, 4, C], bf16, tag="mm")
                for k in range(4):
                    blk = g * 4 + k
                    nc.tensor.transpose(tps4[:, k, :],
                                        SEQ[:, blk * 128:(blk + 1) * 128],
                                        ident[0:C, 0:C])
                mu4 = spool.tile([128, 4], f32, tag="mu4")
                nc.vector.tensor_reduce(mu4[:], tps4[:], Axis.X, Alu.add)
                mun4 = spool.tile([128, 4], f32, tag="mun4")
                nc.vector.tensor_scalar_mul(mun4[:], mu4[:], 1.0 / C)
                h04 = HN0[:, g * 256:(g + 1) * 256].rearrange(
                    "p (b c) -> p b c", b=4)
                nc.vector.tensor_tensor(h04, tps4[:],
                                        mun4[:].to_broadcast((128, 4, C)),
                                        op=Alu.subtract)
                sq4 = spool.tile([128, 4, C], f32, tag="sq4")
                nc.gpsimd.tensor_tensor(sq4[:], h04, h04, op=Alu.mult)
                ssq4 = spool.tile([128, 4], f32, tag="ssq4")
                nc.vector.tensor_reduce(ssq4[:], sq4[:], Axis.X, Alu.add)
                nc.vector.tensor_scalar(VARS[:, g * 4:(g + 1) * 4], ssq4[:],
                                        1.0 / C, EPS,
                                        op0=Alu.mult, op1=Alu.add)
                nc.scalar.activation(SQV[:, g * 4:(g + 1) * 4],
                                     VARS[:, g * 4:(g + 1) * 4], Act.Sqrt)
                nc.vector.reciprocal(RSTD[:, g * 4:(g + 1) * 4],
                                     SQV[:, g * 4:(g + 1) * 4])
            HNT = bpool.tile([128, L // 2], bf16, name="HNT", tag="HNT")
            for g in range(NCH):
                hnT4 = HNT[:, g * 256:(g + 1) * 256].rearrange(
                    "p (b c) -> p b c", b=4)
                nc.gpsimd.tensor_tensor(
                    hnT4, HN0[:, g * 256:(g + 1) * 256].rearrange(
                        "p (b c) -> p b c", b=4),
                    RSTD[:, g * 4:(g + 1) * 4].to_broadcast((128, 4, C)),
                    op=Alu.mult)
                tb4 = ps.tile([C, 4, 128], bf16, tag="mm")
                for k in range(4):
                    blk = g * 4 + k
                    nc.tensor.transpose(tb4[:, k, :],
                                        HNT[:, blk * C:(blk + 1) * C],
                                        ident)
                nc.vector.tensor_scalar(
                    HN[:, g * CH:(g + 1) * CH],
                    tb4[:].rearrange("p a b -> p (a b)"), 0.0,
                    None, op0=Alu.add)

            # ---- in_proj: xm (DVE bias-add) + z (ACT silu) ----
            for chi in range(NCH):
                sl = slice(chi * CH, (chi + 1) * CH)
                xm_ps = ps.tile([DI, CH], f32, tag="mm")
                z_ps = ps.tile([DI, CH], f32, tag="mm")
                nc.tensor.matmul(xm_ps[:], ip_lhsT[0:C, 0:DI], HN[:, sl],
                                 start=True, stop=True)
                nc.tensor.matmul(z_ps[:], ip_lhsT[0:C, DI:2 * DI], HN[:, sl],
                                 start=True, stop=True)
                nc.vector.tensor_scalar(XM0[:, 4 + chi * CH:4 + (chi + 1) * CH],
                                        xm_ps[:], ip_b0, None, op0=Alu.add)
                nc.scalar.activation(ZS[:, sl], z_ps[:], Act.Silu, bias=ip_b1)

            # ---- causal conv1d on PE (4 diag taps) + silu ----
            for chi in range(NCH):
                sl = slice(chi * CH, (chi + 1) * CH)
                cc = ps.tile([DI, CH], f32, tag="mm")
                for tap in range(4):
                    nc.tensor.matmul(cc[:], cdiag[:, tap * 128:(tap + 1) * 128],
                                     XM0[:, 1 + tap + chi * CH:
                                         1 + tap + chi * CH + CH],
                                     start=(tap == 0), stop=(tap == 3))
                nc.scalar.activation(XC[:, sl], cc[:], Act.Silu, bias=cd_b)

            # ---- x_proj: fused dt matmul; B/C rows; esb exp ----
            for chi in range(NCH):
                sl = slice(chi * CH, (chi + 1) * CH)
                dt_ps = ps.tile([DI, CH], f32, tag="mm")
                nc.tensor.matmul(dt_ps[:], M_dt, XC[:, sl],
                                 start=True, stop=True)
                nc.scalar.activation(ESB[:, sl], dt_ps[:], Act.Exp, bias=dt_b)


            # ---- DT = ln(1+esb) in halves (costs ~2 extra act-table loads
            # but lets U/staging start at the front's midpoint);
            # U = DT*XC; stage U/DT to DRAM for tiled replication ----
            LHf = L // 2
            for hf in range(2):
                hsl = slice(hf * LHf, (hf + 1) * LHf)
                nc.scalar.activation(DT[:, hsl], ESB[:, hsl], Act.Ln, bias=1.0)
                nc.vector.tensor_mul(U[:, hsl], DT[:, hsl], XC[:, hsl])
                nc.sync.dma_start(ud_dram[:, hf * LHf:(hf + 1) * LHf],
                                  U[:, hsl])
                nc.sync.dma_start(ud_dram[:, L + hf * LHf:L + (hf + 1) * LHf],
                                  DT[:, hsl])

            # ---- B_tile / C_tile (group-invariant): sel matmul + copy ----
            for chi in range(NCH):
                sl = slice(chi * CH, (chi + 1) * CH)
                bt_ps = ps.tile([DI, CH], f32, tag="mm")
                nc.tensor.matmul(bt_ps[:], W_B, XC[:, sl],
                                 start=True, stop=True)
                nc.scalar.activation(BT[:, sl], bt_ps[:], Act.Copy)
                ct_ps = ps.tile([DI, CH], f32, tag="mm")
                nc.tensor.matmul(ct_ps[:], W_C, XC[:, sl],
                                 start=True, stop=True)
                nc.vector.tensor_scalar(CT[:, sl], ct_ps[:], 0.0, None,
                                        op0=Alu.add)

            # ---- XCD = XC*Dp on Pool (runs during scan phase) ----
            XCD = bpool.tile([DI, L], bf16, name="XCD", tag="ESB")
            for hf in range(2):
                hsl = slice(hf * (L // 2), (hf + 1) * (L // 2))
                nc.gpsimd.tensor_tensor(XCD[:, hsl], XC[:, hsl],
                                        Dp.to_broadcast((DI, L // 2)),
                                        op=Alu.mult)

            # dummy exp: forces the exp act-table load to happen now (ACT
            # idle) instead of being chained onto the first dA exp's waits
            dumex = cpool.tile([128, 1], f32, tag="dumex")
            nc.scalar.activation(dumex[:], BT[:, 0:1], Act.Exp)

            # ---- scan phase: per channel-group g ----
            LH0 = L // 2
            ypsA = [psy.tile([DI, CH], f32, name=f"ypsA{ci}", tag="yps")
                    for ci in range(4)]
            for g in range(NG):
                udt = udpool.tile([DI, 2 * L], bf16, tag="udt")
                for j in range(NS):
                    nc.sync.dma_start(
                        udt[j * 16:(j + 1) * 16, :],
                        ud_dram[g * 16:(g + 1) * 16, :])
                for hf in range(2):
                    hsl = slice(hf * LH0, (hf + 1) * LH0)
                    dA = dapool.tile([DI, LH0], f32, tag="dA")
                    nc.scalar.activation(dA[:],
                                         udt[:, L + hf * LH0:L + (hf + 1) * LH0],
                                         Act.Exp, scale=a_vec[:, g:g + 1])
                    # dbx in-place into the U-half of udt (dead after this)
                    nc.vector.tensor_tensor(udt[:, hsl], udt[:, hsl],
                                            BT[:, hsl], op=Alu.mult)
                    init = 0.0 if hf == 0 else YPs[g][:, LH0 - 1:LH0]
                    nc.vector.tensor_tensor_scan(YPs[g][:, hsl], dA[:],
                                                 udt[:, hsl],
                                                 init, op0=Alu.mult, op1=Alu.add)
                # y partial: YP = H * C_tile (even groups on Pool), split in
                # halves so h0 overlaps the h1 scan and accumulates earlier
                for hf in range(2):
                    hsl2 = slice(hf * LH0, (hf + 1) * LH0)
                    if g != 5 and g != 7:
                        nc.gpsimd.tensor_tensor(YPs[g][:, hsl2], YPs[g][:, hsl2],
                                                CT[:, hsl2], op=Alu.mult)
                    else:
                        nc.vector.tensor_tensor(YPs[g][:, hsl2], YPs[g][:, hsl2],
                                                CT[:, hsl2], op=Alu.mult)
                # incremental y-reduce for chunks 0-3 (PSUM live across phase)
                for ci in range(4):
                    slc = slice(ci * CH, (ci + 1) * CH)
                    nc.tensor.matmul(ypsA[ci][:], Rg[:, g * 128:(g + 1) * 128],
                                     YPs[g][:, slc],
                                     start=(g == 0), stop=(g == NG - 1))

            # ---- y reduce: chunks 0-3 done incrementally; drain + chunks 4-7
            for chi in range(4):
                ysb = spool.tile([DI, CH], bf16, tag="ysb")
                nc.scalar.activation(ysb[:], ypsA[chi][:], Act.Copy)
                nc.sync.dma_start(
                    y_in_t[chi // 2, :, (chi % 2) * CH:(chi % 2 + 1) * CH],
                    ysb[:])
            for chi in range(4, NCH):
                sl = slice(chi * CH, (chi + 1) * CH)
                yps = psy.tile([DI, CH], f32, tag="yps")
                for g in range(NG):
                    nc.tensor.matmul(yps[:], Rg[:, g * 128:(g + 1) * 128],
                                     YPs[g][:, sl],
                                     start=(g == 0), stop=(g == NG - 1))
                ysb = spool.tile([DI, CH], bf16, tag="ysb")
                if chi % 2 == 0:
                    nc.scalar.activation(ysb[:], yps[:], Act.Copy)
                else:
                    nc.vector.tensor_scalar(ysb[:], yps[:], 0.0, None,
                                            op0=Alu.add)
                nc.sync.dma_start(
                    y_in_t[chi // 2, :, (chi % 2) * CH:(chi % 2 + 1) * CH],
                    ysb[:])

            # ---- AllReduce partial y in quarters, pipelined with post/out ----
            # XCD precomputed on Pool (overlaps scan phase)
            YS = bpool.tile([DI, L], bf16, name="YS", tag="HN")
            LQ = L // 4
            for q in range(4):
                qsl = slice(q * LQ, (q + 1) * LQ)
                if sim:
                    nc.sync.dma_start(y_out_t[q], y_in_t[q])
                else:
                    nc.gpsimd.collective_compute(
                        "AllReduce", Alu.add, replica_groups=groups,
                        ins=[y_in_t[q].opt()], outs=[y_out_t[q].opt()])
                nc.sync.dma_start(YSUM[:, qsl], y_out_t[q])
                nc.vector.tensor_add(XCD[:, qsl], YSUM[:, qsl], XCD[:, qsl])
                nc.vector.tensor_mul(YS[:, qsl], XCD[:, qsl], ZS[:, qsl])
                for ci in range(2):
                    chi = q * 2 + ci
                    sl = slice(chi * CH, (chi + 1) * CH)
                    op_ps = psy.tile([C, CH], f32, tag="yps")
                    nc.tensor.matmul(op_ps[:], op_lhsT, YS[:, sl],
                                     start=True, stop=True)
                    outc = spool.tile([C, CH], f32, tag="outc")
                    nc.vector.tensor_tensor(outc[:], op_ps[:], SEQ[:, sl],
                                            op=Alu.add)
                    nc.sync.dma_start(out_d[:, sl], outc[:])

    nc.compile()
    return nc


def _host_precompute(inp):
    import ml_dtypes
    f = lambda k: np.asarray(inp[k], np.float32)
    bf = lambda a: np.ascontiguousarray(a.astype(ml_dtypes.bfloat16))
    w1 = f("conv_w")[:, :, 0, 0]
    wh = f("dwh_w")[:, 0, :, 0]
    ww = f("dww_w")[:, 0, 0, :]
    s_bn = f("bn_g") / np.sqrt(f("bn_v") + EPS)
    taps = [
        w1 * (1.0 + wh[:, 1] + ww[:, 1])[None, :],   # center
        w1 * wh[:, 0][None, :],                       # up
        w1 * wh[:, 2][None, :],                       # down
        w1 * ww[:, 0][None, :],                       # left
        w1 * ww[:, 2][None, :],                       # right
    ]
    cw = np.concatenate([t.T for t in taps], axis=1)
    cw = cw * np.tile(s_bn, 5)[None, :]  # [cin=64, 5*64]
    btot = f("conv_b") + w1 @ (f("dwh_b") + f("dww_b"))
    bn_bias = s_bn * (btot - f("bn_m")) + f("bn_b")
    ipw = f("in_proj_w")
    ip_lhsT = (ipw * f("ln_g")[None, :]).T            # [64, 256]
    ip_bias = ipw @ f("ln_b")                          # [256]
    xpw = f("x_proj_w")                                # [36, 128]
    M_dt = f("dt_proj_w") @ xpw[:DR]                   # [128, 128]
    a_full = -np.exp(np.asarray(inp["A_log"], np.float32))  # [DI, DS]
    cdw = f("convd_w")[:, 0, :]                        # [128, 4]

    per_sigma = []
    for sg in range(2):
        s_lo = sg * NS
        cf32 = np.zeros((128, 32), np.float32)
        cf32[:C, 0] = s_bn
        cf32[:C, 1] = bn_bias
        cf32[:, 2] = ip_bias[:DI]
        cf32[:, 3] = ip_bias[DI:]
        cf32[:, 4] = f("convd_b")
        cf32[:, 5] = f("dt_proj_b")
        cf32[:, 6] = f("Dp")
        # a_vec per group g: a[p] = a_full[16g + p%16, s_lo + p//16]
        p = np.arange(128)
        for g in range(NG):
            cf32[:, 8 + g] = a_full[16 * g + p % 16, s_lo + p // 16]

        cbf = np.zeros((128, 2688), np.float32)
        cbf[:, 0:128] = np.eye(128, dtype=np.float32)
        cbf[:C, 128:448] = cw
        cbf[:C, 448:704] = ip_lhsT
        cbf[:, 704:832] = M_dt.T
        for tap in range(4):
            cbf[:, 832 + tap * 128:832 + (tap + 1) * 128] = np.diag(cdw[:, tap])
        # fused B/C broadcast: W_B[p, :] = xpw_B[s_lo + p//16, :] (stored T)
        for pp in range(128):
            cbf[:, 1344 + pp] = xpw[DR + s_lo + pp // 16]
            cbf[:, 1472 + pp] = xpw[DR + DS + s_lo + pp // 16]
        # Rg: R_g[p, d] = 1 iff d == 16g + p%16
        for g in range(NG):
            for pp in range(128):
                cbf[pp, 1600 + g * 128 + 16 * g + pp % 16] = 1.0
        cbf[:, 2624:2688] = f("out_proj_w").T
        per_sigma.append(dict(cf32=cf32, cbf=bf(cbf)))
    return {}, per_sigma


def _shift_images(xb):
    # 5 pre-shifted copies: ctr, up(reads h-1), dn(h+1), lf(w-1), rt(w+1)
    import ml_dtypes
    out = np.zeros((C, 5, H, W), np.float32)
    out[:, 0] = xb
    out[:, 1, 1:, :] = xb[:, :-1, :]
    out[:, 2, :-1, :] = xb[:, 1:, :]
    out[:, 3, :, 1:] = xb[:, :, :-1]
    out[:, 4, :, :-1] = xb[:, :, 1:]
    return np.ascontiguousarray(
        out.transpose(1, 0, 2, 3).reshape(5, C, L).transpose(1, 0, 2)
        .reshape(C, 5 * L).astype(ml_dtypes.bfloat16))


TRACE = False
LAST_EXEC_NS = None
LAST_TRACE_DIR = None


def kernel(**inputs):
    global LAST_EXEC_NS, LAST_TRACE_DIR
    from concourse.bass_utils import run_bass_kernel_spmd

    if "nc" not in _cached:
        _cached["nc"] = _build_program()
    nc = _cached["nc"]

    common, per_sigma = _host_precompute(inputs)
    x = np.asarray(inputs["x"], np.float32)
    in_maps = []
    for c in range(NCORES):
        b, sg = c // 2, c % 2
        m = dict(common)
        m.update(per_sigma[sg])
        m["ximgs"] = _shift_images(x[b])
        in_maps.append(m)

    kw = {}
    if TRACE:
        import tempfile
        LAST_TRACE_DIR = tempfile.mkdtemp(prefix="bass_trace_")
        kw = dict(trace=True, tmpdir=LAST_TRACE_DIR)
    r = run_bass_kernel_spmd(nc, in_maps, list(range(NCORES)), **kw)
    if r.exec_time_ns is not None:
        LAST_EXEC_NS = r.exec_time_ns
    res = r.results
    out = np.empty((B, C, H, W), np.float32)
    for b in range(B):
        out[b] = np.asarray(res[2 * b]["out_f"], np.float32).reshape(C, H, W)
    return out


# revision 45
# speedup vs baseline: 1.0445x; 1.0069x over previous
"""Trainium2 Bass kernel v2 for nn_DecoderBlock_Mamba.

Sharding: 8 cores = (batch b in 0..3) x (state-half sigma in {0,1}).
Scan uses a tiled partition layout: partition p = j*16 + i holds state
(s_lo + j) and channel-group offset i; 8 channel-groups g cover d = 16g + i.
This makes the B/C broadcasts group-invariant (built once) and both dbx / y
multiplies all-SBUF-bf16 (2x DVE fast path). U/DT are staged to DRAM and
replicated into the tiled layout by 8 DMAs per group (DMA engines are idle).

Self-contained: hardcodes all shapes; no sibling imports.
"""
import numpy as np

C = 64
DI = 128
DS = 16
DR = 4
B = 4
H = 64
W = 64
L = H * W
NS = 8            # states per core
NG = 8            # channel groups (of 16) per core
NCORES = 8
NCH = 8           # L chunks of 512
CH = 512
EPS = 1e-5

_cached = {}


def _build_program(sim=False, phases=3):
    import concourse.bass as bass
    import concourse.bacc as bacc
    import concourse.mybir as mybir
    import concourse.tile as tile

    dt = mybir.dt
    f32 = dt.float32
    bf16 = dt.bfloat16
    Act = mybir.ActivationFunctionType
    Alu = mybir.AluOpType
    Axis = mybir.AxisListType

    nc = bacc.Bacc(None, target_bir_lowering=False)

    def din(name, shape, dtype=f32):
        return nc.dram_tensor(name, shape, dtype, kind="ExternalInput")

    ximgs_d = din("ximgs", [C, 5 * L], bf16)
    cf32_d = din("cf32", [128, 32])
    cbf_d = din("cbf", [128, 2688], bf16)

    out_d = nc.dram_tensor("out_f", [C, L], f32, kind="ExternalOutput")

    groups = [[0, 1], [2, 3], [4, 5], [6, 7]]

    with tile.TileContext(nc) as tc:
        with (
            tc.tile_pool(name="dram", bufs=1, space="DRAM") as dpool,
            tc.tile_pool(name="const", bufs=1) as cpool,
            tc.tile_pool(name="big", bufs=1) as bpool,
            tc.tile_pool(name="sm", bufs=2) as spool,
            tc.tile_pool(name="ud", bufs=2) as udpool,
            tc.tile_pool(name="da", bufs=2) as dapool,
            tc.tile_pool(name="ps", bufs=4, space="PSUM") as ps,
            tc.tile_pool(name="psy", bufs=4, space="PSUM") as psy,
        ):
            # ---- constants (packed: 2 DMAs) ----
            cf = cpool.tile([128, 32], f32)
            cb = cpool.tile([128, 2688], bf16)
            nc.sync.dma_start(cf[:], cf32_d[:])
            nc.sync.dma_start(cb[:], cbf_d[:])
            bn_s = cf[0:C, 0:1]
            bn_b = cf[0:C, 1:2]
            ip_b0 = cf[:, 2:3]
            ip_b1 = cf[:, 3:4]
            cd_b = cf[:, 4:5]
            dt_b = cf[:, 5:6]
            Dp = cf[:, 6:7]
            a_vec = cf[:, 8:16]          # per-group a scale [128, 8]

            ident = cb[:, 0:128]
            cw = cb[0:C, 128:448]
            ip_lhsT = cb[0:C, 448:704]
            M_dt = cb[:, 704:832]
            cdiag = cb[:, 832:1344]      # 4 diag taps [128, 4*128]
            W_B = cb[:, 1344:1472]       # fused B broadcast [128,128]
            W_C = cb[:, 1472:1600]       # fused C broadcast
            Rg = cb[:, 1600:2624]        # 8 x [128,128] reduce mats
            op_lhsT = cb[:, 2624:2688]

            # ---- persistent activations ----
            SEQ = bpool.tile([C, L], bf16)            # BN+ReLU out (residual)
            HN = bpool.tile([C, L], bf16)             # LN-normalized
            XM0 = bpool.tile([DI, L + 4], bf16, name="XM0", tag="YP5")       # conv1d in, data @ col 4
            ZS = bpool.tile([DI, L], bf16)            # silu(z)
            XC = bpool.tile([DI, L], bf16)
            ESB = bpool.tile([DI, L], bf16, tag="ESB")
            UD = bpool.tile([DI, 2 * L], bf16, name="UD", tag="HN")
            BT = bpool.tile([DI, L], bf16, name="BT")      # B_tile (j slow)
            CT = bpool.tile([DI, L], bf16, name="CT")      # C_tile
            YPs = [bpool.tile([DI, L], bf16, name=f"YP{g}", tag=f"YP{g}")
                   for g in range(NG)]
            YSUM = bpool.tile([DI, L], bf16, name="YSUM", tag="YSUM")

            y_in_t = dpool.tile([4, DI, L // 4], bf16, tag="yin")
            y_out_t = dpool.tile([4, DI, L // 4], bf16, tag="yout")

            # Prime ACT's vector clock on the const DMAs
            warm = cpool.tile([128, 1], f32, tag="warm")
            nc.scalar.activation(warm[:], cf[:, 0:1], Act.Copy)
            warm2 = cpool.tile([128, 1], bf16, tag="warm2")
            nc.scalar.activation(warm2[:], cb[:, 0:1], Act.Copy)
            nc.vector.tensor_scalar_mul(XM0[:, 0:4], cf[:, 0:4], 0.0)

            IMGS = [bpool.tile([C, L], bf16, name=f"img{t}", tag=f"YP{t}")
                    for t in range(5)]
            for t in range(5):
                nc.sync.dma_start(IMGS[t][:], ximgs_d[:, t * L:(t + 1) * L])

            # ---- front conv: 5 accumulating taps + BN + ReLU ----
            for chi in range(NCH):
                sl = slice(chi * CH, (chi + 1) * CH)
                pc = ps.tile([C, CH], f32, tag="mm")
                for tap in range(5):
                    nc.tensor.matmul(pc[:], cw[:, tap * C:(tap + 1) * C],
                                     IMGS[tap][:, sl],
                                     start=(tap == 0), stop=(tap == 4))
                nc.scalar.activation(SEQ[:, sl], pc[:],
                                     Act.Relu, bias=bn_b)

            # ---- LayerNorm over channels, batched 4 blocks per op ----
            HN0 = bpool.tile([128, L // 2], bf16, name="HN0", tag="HN0")
            VARS = spool.tile([128, 32], f32, tag="VARS")
            SQV = spool.tile([128, 32], f32, tag="SQV")
            RSTD = spool.tile([128, 32], f32, tag="RSTD")
            for g in range(NCH):
                tps4 = ps.tile([128, 4, C], bf16, tag="mm")
                for k in range(4):
                    blk = g * 4 + k
                    nc.tensor.transpose(tps4[:, k, :],
                                        SEQ[:, blk * 128:(blk + 1) * 128],
                                        ident[0:C, 0:C])
                mu4 = spool.tile([128, 4], f32, tag="mu4")
                nc.vector.tensor_reduce(mu4[:], tps4[:], Axis.X, Alu.add)
                mun4 = spool.tile([128, 4], f32, tag="mun4")
                nc.vector.tensor_scalar_mul(mun4[:], mu4[:], 1.0 / C)
                h04 = HN0[:, g * 256:(g + 1) * 256].rearrange(
                    "p (b c) -> p b c", b=4)
                nc.vector.tensor_tensor(h04, tps4[:],
                                        mun4[:].to_broadcast((128, 4, C)),
                                        op=Alu.subtract)
                sq4 = spool.tile([128, 4, C], f32, tag="sq4")
                nc.gpsimd.tensor_tensor(sq4[:], h04, h04, op=Alu.mult)
                ssq4 = spool.tile([128, 4], f32, tag="ssq4")
                nc.vector.tensor_reduce(ssq4[:], sq4[:], Axis.X, Alu.add)
                nc.vector.tensor_scalar(VARS[:, g * 4:(g + 1) * 4], ssq4[:],
                                        1.0 / C, EPS,
                                        op0=Alu.mult, op1=Alu.add)
                nc.scalar.activation(SQV[:, g * 4:(g + 1) * 4],
                                     VARS[:, g * 4:(g + 1) * 4], Act.Sqrt)
                nc.vector.reciprocal(RSTD[:, g * 4:(g + 1) * 4],
                                     SQV[:, g * 4:(g + 1) * 4])
            HNT = bpool.tile([128, L // 2], bf16, name="HNT", tag="HNT")
            for g in range(NCH):
                hnT4 = HNT[:, g * 256:(g + 1) * 256].rearrange(
                    "p (b c) -> p b c", b=4)
                nc.gpsimd.tensor_tensor(
                    hnT4, HN0[:, g * 256:(g + 1) * 256].rearrange(
                        "p (b c) -> p b c", b=4),
                    RSTD[:, g * 4:(g + 1) * 4].to_broadcast((128, 4, C)),
                    op=Alu.mult)
                tb4 = ps.tile([C, 4, 128], bf16, tag="mm")
                for k in range(4):
                    blk = g * 4 + k
                    nc.tensor.transpose(tb4[:, k, :],
                                        HNT[:, blk * C:(blk + 1) * C],
                                        ident)
                nc.vector.tensor_scalar(
                    HN[:, g * CH:(g + 1) * CH],
                    tb4[:].rearrange("p a b -> p (a b)"), 0.0,
                    None, op0=Alu.add)

            # ---- in_proj: xm (DVE bias-add) + z (ACT silu) ----
            for chi in range(NCH):
                sl = slice(chi * CH, (chi + 1) * CH)
                xm_ps = ps.tile([DI, CH], f32, tag="mm")
                z_ps = ps.tile([DI, CH], f32, tag="mm")
                nc.tensor.matmul(xm_ps[:], ip_lhsT[0:C, 0:DI], HN[:, sl],
                                 start=True, stop=True)
                nc.tensor.matmul(z_ps[:], ip_lhsT[0:C, DI:2 * DI], HN[:, sl],
                                 start=True, stop=True)
                nc.vector.tensor_scalar(XM0[:, 4 + chi * CH:4 + (chi + 1) * CH],
                                        xm_ps[:], ip_b0, None, op0=Alu.add)
                nc.scalar.activation(ZS[:, sl], z_ps[:], Act.Silu, bias=ip_b1)

            # ---- causal conv1d on PE (4 diag taps) + silu ----
            for chi in range(NCH):
                sl = slice(chi * CH, (chi + 1) * CH)
                cc = ps.tile([DI, CH], f32, tag="mm")
                for tap in range(4):
                    nc.tensor.matmul(cc[:], cdiag[:, tap * 128:(tap + 1) * 128],
                                     XM0[:, 1 + tap + chi * CH:
                                         1 + tap + chi * CH + CH],
                                     start=(tap == 0), stop=(tap == 3))
                nc.scalar.activation(XC[:, sl], cc[:], Act.Silu, bias=cd_b)

            # ---- x_proj: fused dt matmul; B/C rows; esb exp ----
            for chi in range(NCH):
                sl = slice(chi * CH, (chi + 1) * CH)
                dt_ps = ps.tile([DI, CH], f32, tag="mm")
                nc.tensor.matmul(dt_ps[:], M_dt, XC[:, sl],
                                 start=True, stop=True)
                nc.scalar.activation(ESB[:, sl], dt_ps[:], Act.Exp, bias=dt_b)


            # ---- DT = ln(1+esb) in halves (costs ~2 extra act-table loads
            # but lets U/staging start at the front's midpoint);
            # U = DT*XC; stage U/DT to DRAM for tiled replication ----
            LHf = L // 2
            for hf in range(2):
                hsl = slice(hf * LHf, (hf + 1) * LHf)
                dsl = slice(L + hf * LHf, L + (hf + 1) * LHf)
                nc.scalar.activation(UD[:, dsl], ESB[:, hsl], Act.Ln, bias=1.0)
                nc.vector.tensor_mul(UD[:, hsl], UD[:, dsl], XC[:, hsl])

            # ---- B_tile / C_tile (group-invariant): sel matmul + copy ----
            for chi in range(NCH):
                sl = slice(chi * CH, (chi + 1) * CH)
                bt_ps = ps.tile([DI, CH], f32, tag="mm")
                nc.tensor.matmul(bt_ps[:], W_B, XC[:, sl],
                                 start=True, stop=True)
                nc.scalar.activation(BT[:, sl], bt_ps[:], Act.Copy)
                ct_ps = ps.tile([DI, CH], f32, tag="mm")
                nc.tensor.matmul(ct_ps[:], W_C, XC[:, sl],
                                 start=True, stop=True)
                nc.vector.tensor_scalar(CT[:, sl], ct_ps[:], 0.0, None,
                                        op0=Alu.add)

            # ---- XCD = XC*Dp on Pool (runs during scan phase) ----
            XCD = bpool.tile([DI, L], bf16, name="XCD", tag="ESB")
            for hf in range(2):
                hsl = slice(hf * (L // 2), (hf + 1) * (L // 2))
                nc.gpsimd.tensor_tensor(XCD[:, hsl], XC[:, hsl],
                                        Dp.to_broadcast((DI, L // 2)),
                                        op=Alu.mult)

            # dummy exp: forces the exp act-table load to happen now (ACT
            # idle) instead of being chained onto the first dA exp's waits
            dumex = cpool.tile([128, 1], f32, tag="dumex")
            nc.scalar.activation(dumex[:], BT[:, 0:1], Act.Exp)

            # ---- scan phase: per channel-group g ----
            LH0 = L // 2
            ypsA = [psy.tile([DI, CH], f32, name=f"ypsA{ci}", tag="yps")
                    for ci in range(4)]
            for g in range(NG):
                udt = udpool.tile([DI, 2 * L], bf16, tag="udt")
                for j in range(NS):
                    nc.sync.dma_start(
                        udt[j * 16:(j + 1) * 16, :],
                        UD[g * 16:(g + 1) * 16, :])
                LQ0 = L // 4
                for hf in range(2):
                    hsl = slice(hf * LH0, (hf + 1) * LH0)
                    # dbx in-place into the U-half of udt (dead after this)
                    nc.vector.tensor_tensor(udt[:, hsl], udt[:, hsl],
                                            BT[:, hsl], op=Alu.mult)
                    for qq in range(2):
                        qf = hf * 2 + qq
                        qsl = slice(qf * LQ0, (qf + 1) * LQ0)
                        dA = dapool.tile([DI, LQ0], f32, tag="dA")
                        nc.scalar.activation(
                            dA[:], udt[:, L + qf * LQ0:L + (qf + 1) * LQ0],
                            Act.Exp, scale=a_vec[:, g:g + 1])
                        init = (0.0 if qf == 0
                                else YPs[g][:, qf * LQ0 - 1:qf * LQ0])
                        nc.vector.tensor_tensor_scan(YPs[g][:, qsl], dA[:],
                                                     udt[:, qsl], init,
                                                     op0=Alu.mult,
                                                     op1=Alu.add)
                # y partial: YP = H * C_tile (even groups on Pool), split in
                # halves so h0 overlaps the h1 scan and accumulates earlier
                for hf in range(2):
                    hsl2 = slice(hf * LH0, (hf + 1) * LH0)
                    if g != 5 and g != 7:
                        nc.gpsimd.tensor_tensor(YPs[g][:, hsl2], YPs[g][:, hsl2],
                                                CT[:, hsl2], op=Alu.mult)
                    else:
                        nc.vector.tensor_tensor(YPs[g][:, hsl2], YPs[g][:, hsl2],
                                                CT[:, hsl2], op=Alu.mult)
                # incremental y-reduce for chunks 0-3 (PSUM live across phase)
                for ci in range(4):
                    slc = slice(ci * CH, (ci + 1) * CH)
                    nc.tensor.matmul(ypsA[ci][:], Rg[:, g * 128:(g + 1) * 128],
                                     YPs[g][:, slc],
                                     start=(g == 0), stop=(g == NG - 1))

            # ---- y reduce: chunks 0-3 done incrementally; drain + chunks 4-7
            for chi in range(4):
                ysb = spool.tile([DI, CH], bf16, tag="ysb")
                nc.scalar.activation(ysb[:], ypsA[chi][:], Act.Copy)
                nc.sync.dma_start(
                    y_in_t[chi // 2, :, (chi % 2) * CH:(chi % 2 + 1) * CH],
                    ysb[:])
            for chi in range(4, NCH):
                sl = slice(chi * CH, (chi + 1) * CH)
                yps = psy.tile([DI, CH], f32, tag="yps")
                for g in range(NG):
                    nc.tensor.matmul(yps[:], Rg[:, g * 128:(g + 1) * 128],
                                     YPs[g][:, sl],
                                     start=(g == 0), stop=(g == NG - 1))
                ysb = spool.tile([DI, CH], bf16, tag="ysb")
                if chi % 2 == 0:
                    nc.scalar.activation(ysb[:], yps[:], Act.Copy)
                else:
                    nc.vector.tensor_scalar(ysb[:], yps[:], 0.0, None,
                                            op0=Alu.add)
                nc.sync.dma_start(
                    y_in_t[chi // 2, :, (chi % 2) * CH:(chi % 2 + 1) * CH],
                    ysb[:])

            # ---- AllReduce partial y in quarters, pipelined with post/out ----
            # XCD precomputed on Pool (overlaps scan phase)
            YS = bpool.tile([DI, L], bf16, name="YS", tag="HN")
            LQ = L // 4
            for q in range(4):
                qsl = slice(q * LQ, (q + 1) * LQ)
                if sim:
                    nc.sync.dma_start(y_out_t[q], y_in_t[q])
                else:
                    nc.gpsimd.collective_compute(
                        "AllReduce", Alu.add, replica_groups=groups,
                        ins=[y_in_t[q].opt()], outs=[y_out_t[q].opt()])
                nc.sync.dma_start(YSUM[:, qsl], y_out_t[q])
                nc.vector.tensor_add(XCD[:, qsl], YSUM[:, qsl], XCD[:, qsl])
                nc.vector.tensor_mul(YS[:, qsl], XCD[:, qsl], ZS[:, qsl])
                for ci in range(2):
                    chi = q * 2 + ci
                    sl = slice(chi * CH, (chi + 1) * CH)
                    op_ps = psy.tile([C, CH], f32, tag="yps")
                    nc.tensor.matmul(op_ps[:], op_lhsT, YS[:, sl],
                                     start=True, stop=True)
                    outc = spool.tile([C, CH], f32, tag="outc")
                    nc.vector.tensor_tensor(outc[:], op_ps[:], SEQ[:, sl],
                                            op=Alu.add)
                    nc.sync.dma_start(out_d[:, sl], outc[:])

    nc.compile()
    return nc


def _host_precompute(inp):
    import ml_dtypes
    f = lambda k: np.asarray(inp[k], np.float32)
    bf = lambda a: np.ascontiguousarray(a.astype(ml_dtypes.bfloat16))
    w1 = f("conv_w")[:, :, 0, 0]
    wh = f("dwh_w")[:, 0, :, 0]
    ww = f("dww_w")[:, 0, 0, :]
    s_bn = f("bn_g") / np.sqrt(f("bn_v") + EPS)
    taps = [
        w1 * (1.0 + wh[:, 1] + ww[:, 1])[None, :],   # center
        w1 * wh[:, 0][None, :],                       # up
        w1 * wh[:, 2][None, :],                       # down
        w1 * ww[:, 0][None, :],                       # left
        w1 * ww[:, 2][None, :],                       # right
    ]
    cw = np.concatenate([t.T for t in taps], axis=1)
    cw = cw * np.tile(s_bn, 5)[None, :]  # [cin=64, 5*64]
    btot = f("conv_b") + w1 @ (f("dwh_b") + f("dww_b"))
    bn_bias = s_bn * (btot - f("bn_m")) + f("bn_b")
    ipw = f("in_proj_w")
    ip_lhsT = (ipw * f("ln_g")[None, :]).T            # [64, 256]
    ip_bias = ipw @ f("ln_b")                          # [256]
    xpw = f("x_proj_w")                                # [36, 128]
    M_dt = f("dt_proj_w") @ xpw[:DR]                   # [128, 128]
    a_full = -np.exp(np.asarray(inp["A_log"], np.float32))  # [DI, DS]
    cdw = f("convd_w")[:, 0, :]                        # [128, 4]

    per_sigma = []
    for sg in range(2):
        s_lo = sg * NS
        cf32 = np.zeros((128, 32), np.float32)
        cf32[:C, 0] = s_bn
        cf32[:C, 1] = bn_bias
        cf32[:, 2] = ip_bias[:DI]
        cf32[:, 3] = ip_bias[DI:]
        cf32[:, 4] = f("convd_b")
        cf32[:, 5] = f("dt_proj_b")
        cf32[:, 6] = f("Dp")
        # a_vec per group g: a[p] = a_full[16g + p%16, s_lo + p//16]
        p = np.arange(128)
        for g in range(NG):
            cf32[:, 8 + g] = a_full[16 * g + p % 16, s_lo + p // 16]

        cbf = np.zeros((128, 2688), np.float32)
        cbf[:, 0:128] = np.eye(128, dtype=np.float32)
        cbf[:C, 128:448] = cw
        cbf[:C, 448:704] = ip_lhsT
        cbf[:, 704:832] = M_dt.T
        for tap in range(4):
            cbf[:, 832 + tap * 128:832 + (tap + 1) * 128] = np.diag(cdw[:, tap])
        # fused B/C broadcast: W_B[p, :] = xpw_B[s_lo + p//16, :] (stored T)
        for pp in range(128):
            cbf[:, 1344 + pp] = xpw[DR + s_lo + pp // 16]
            cbf[:, 1472 + pp] = xpw[DR + DS + s_lo + pp // 16]
        # Rg: R_g[p, d] = 1 iff d == 16g + p%16
        for g in range(NG):
            for pp in range(128):
                cbf[pp, 1600 + g * 128 + 16 * g + pp % 16] = 1.0
        cbf[:, 2624:2688] = f("out_proj_w").T
        per_sigma.append(dict(cf32=cf32, cbf=bf(cbf)))
    return {}, per_sigma


def _shift_images(xb):
    # 5 pre-shifted copies: ctr, up(reads h-1), dn(h+1), lf(w-1), rt(w+1)
    import ml_dtypes
    out = np.zeros((C, 5, H, W), np.float32)
    out[:, 0] = xb
    out[:, 1, 1:, :] = xb[:, :-1, :]
    out[:, 2, :-1, :] = xb[:, 1:, :]
    out[:, 3, :, 1:] = xb[:, :, :-1]
    out[:, 4, :, :-1] = xb[:, :, 1:]
    return np.ascontiguousarray(
        out.transpose(1, 0, 2, 3).reshape(5, C, L).transpose(1, 0, 2)
        .reshape(C, 5 * L).astype(ml_dtypes.bfloat16))


TRACE = False
LAST_EXEC_NS = None
LAST_TRACE_DIR = None


def kernel(**inputs):
    global LAST_EXEC_NS, LAST_TRACE_DIR
    from concourse.bass_utils import run_bass_kernel_spmd

    if "nc" not in _cached:
        _cached["nc"] = _build_program()
    nc = _cached["nc"]

    common, per_sigma = _host_precompute(inputs)
    x = np.asarray(inputs["x"], np.float32)
    in_maps = []
    for c in range(NCORES):
        b, sg = c // 2, c % 2
        m = dict(common)
        m.update(per_sigma[sg])
        m["ximgs"] = _shift_images(x[b])
        in_maps.append(m)

    kw = {}
    if TRACE:
        import tempfile
        LAST_TRACE_DIR = tempfile.mkdtemp(prefix="bass_trace_")
        kw = dict(trace=True, tmpdir=LAST_TRACE_DIR)
    r = run_bass_kernel_spmd(nc, in_maps, list(range(NCORES)), **kw)
    if r.exec_time_ns is not None:
        LAST_EXEC_NS = r.exec_time_ns
    res = r.results
    out = np.empty((B, C, H, W), np.float32)
    for b in range(B):
        out[b] = np.asarray(res[2 * b]["out_f"], np.float32).reshape(C, H, W)
    return out


# revision 46
# speedup vs baseline: 1.0743x; 1.0285x over previous
"""Trainium2 Bass kernel v2 for nn_DecoderBlock_Mamba.

Sharding: 8 cores = (batch b in 0..3) x (state-half sigma in {0,1}).
Scan uses a tiled partition layout: partition p = j*16 + i holds state
(s_lo + j) and channel-group offset i; 8 channel-groups g cover d = 16g + i.
This makes the B/C broadcasts group-invariant (built once) and both dbx / y
multiplies all-SBUF-bf16 (2x DVE fast path). U/DT are staged to DRAM and
replicated into the tiled layout by 8 DMAs per group (DMA engines are idle).

Self-contained: hardcodes all shapes; no sibling imports.
"""
import numpy as np

C = 64
DI = 128
DS = 16
DR = 4
B = 4
H = 64
W = 64
L = H * W
NS = 8            # states per core
NG = 8            # channel groups (of 16) per core
NCORES = 8
NCH = 8           # L chunks of 512
CH = 512
EPS = 1e-5

_cached = {}


def _build_program(sim=False, phases=3):
    import concourse.bass as bass
    import concourse.bacc as bacc
    import concourse.mybir as mybir
    import concourse.tile as tile

    dt = mybir.dt
    f32 = dt.float32
    bf16 = dt.bfloat16
    Act = mybir.ActivationFunctionType
    Alu = mybir.AluOpType
    Axis = mybir.AxisListType

    nc = bacc.Bacc(None, target_bir_lowering=False)

    def din(name, shape, dtype=f32):
        return nc.dram_tensor(name, shape, dtype, kind="ExternalInput")

    ximgs_d = din("ximgs", [C, 5 * L], bf16)
    cf32_d = din("cf32", [128, 32])
    cbf_d = din("cbf", [128, 2688], bf16)

    out_d = nc.dram_tensor("out_f", [C, L], f32, kind="ExternalOutput")

    groups = [[0, 1], [2, 3], [4, 5], [6, 7]]

    with tile.TileContext(nc) as tc:
        with (
            tc.tile_pool(name="dram", bufs=1, space="DRAM") as dpool,
            tc.tile_pool(name="const", bufs=1) as cpool,
            tc.tile_pool(name="big", bufs=1) as bpool,
            tc.tile_pool(name="sm", bufs=2) as spool,
            tc.tile_pool(name="ud", bufs=2) as udpool,
            tc.tile_pool(name="da", bufs=1) as dapool,
            tc.tile_pool(name="ps", bufs=4, space="PSUM") as ps,
            tc.tile_pool(name="psy", bufs=4, space="PSUM") as psy,
        ):
            # ---- constants (packed: 2 DMAs) ----
            cf = cpool.tile([128, 32], f32)
            cb = cpool.tile([128, 2688], bf16)
            nc.sync.dma_start(cf[:], cf32_d[:])
            nc.sync.dma_start(cb[:], cbf_d[:])
            bn_s = cf[0:C, 0:1]
            bn_b = cf[0:C, 1:2]
            ip_b0 = cf[:, 2:3]
            ip_b1 = cf[:, 3:4]
            cd_b = cf[:, 4:5]
            dt_b = cf[:, 5:6]
            Dp = cf[:, 6:7]
            a_vec = cf[:, 8:16]          # per-group a scale [128, 8]

            ident = cb[:, 0:128]
            cw = cb[0:C, 128:448]
            ip_lhsT = cb[0:C, 448:704]
            M_dt = cb[:, 704:832]
            cdiag = cb[:, 832:1344]      # 4 diag taps [128, 4*128]
            W_B = cb[:, 1344:1472]       # fused B broadcast [128,128]
            W_C = cb[:, 1472:1600]       # fused C broadcast
            Rg = cb[:, 1600:2624]        # 8 x [128,128] reduce mats
            op_lhsT = cb[:, 2624:2688]

            # ---- persistent activations ----
            SEQ = bpool.tile([C, L], bf16)            # BN+ReLU out (residual)
            HN = bpool.tile([C, L], bf16)             # LN-normalized
            XM0 = bpool.tile([DI, L + 4], bf16, name="XM0", tag="YP5")       # conv1d in, data @ col 4
            ZS = bpool.tile([DI, L], bf16)            # silu(z)
            XC = bpool.tile([DI, L], bf16)
            ESB = bpool.tile([DI, L], bf16, tag="ESB")
            UD = bpool.tile([DI, 2 * L], bf16, name="UD", tag="HN")
            BT = bpool.tile([DI, L], bf16, name="BT")      # B_tile (j slow)
            CT = bpool.tile([DI, L], bf16, name="CT")      # C_tile
            YPs = [bpool.tile([DI, L], bf16, name=f"YP{g}", tag=f"YP{g}")
                   for g in range(NG)]
            YSUM = bpool.tile([DI, L], bf16, name="YSUM", tag="YSUM")

            y_in_t = dpool.tile([4, DI, L // 4], bf16, tag="yin")
            y_out_t = dpool.tile([4, DI, L // 4], bf16, tag="yout")

            # Prime ACT's vector clock on the const DMAs
            warm = cpool.tile([128, 1], f32, tag="warm")
            nc.scalar.activation(warm[:], cf[:, 0:1], Act.Copy)
            warm2 = cpool.tile([128, 1], bf16, tag="warm2")
            nc.scalar.activation(warm2[:], cb[:, 0:1], Act.Copy)
            nc.vector.tensor_scalar_mul(XM0[:, 0:4], cf[:, 0:4], 0.0)

            IMGS = [bpool.tile([C, L], bf16, name=f"img{t}", tag=f"YP{t}")
                    for t in range(5)]
            for t in range(5):
                nc.sync.dma_start(IMGS[t][:], ximgs_d[:, t * L:(t + 1) * L])

            # ---- front conv: 5 accumulating taps + BN + ReLU ----
            for chi in range(NCH):
                sl = slice(chi * CH, (chi + 1) * CH)
                pc = ps.tile([C, CH], f32, tag="mm")
                for tap in range(5):
                    nc.tensor.matmul(pc[:], cw[:, tap * C:(tap + 1) * C],
                                     IMGS[tap][:, sl],
                                     start=(tap == 0), stop=(tap == 4))
                nc.scalar.activation(SEQ[:, sl], pc[:],
                                     Act.Relu, bias=bn_b)

            # ---- LayerNorm over channels, batched 4 blocks per op ----
            HN0 = bpool.tile([128, L // 2], bf16, name="HN0", tag="HN0")
            VARS = spool.tile([128, 32], f32, tag="VARS")
            SQV = spool.tile([128, 32], f32, tag="SQV")
            RSTD = spool.tile([128, 32], f32, tag="RSTD")
            for g in range(NCH):
                tps4 = ps.tile([128, 4, C], bf16, tag="mm")
                for k in range(4):
                    blk = g * 4 + k
                    nc.tensor.transpose(tps4[:, k, :],
                                        SEQ[:, blk * 128:(blk + 1) * 128],
                                        ident[0:C, 0:C])
                mu4 = spool.tile([128, 4], f32, tag="mu4")
                nc.vector.tensor_reduce(mu4[:], tps4[:], Axis.X, Alu.add)
                mun4 = spool.tile([128, 4], f32, tag="mun4")
                nc.vector.tensor_scalar_mul(mun4[:], mu4[:], 1.0 / C)
                h04 = HN0[:, g * 256:(g + 1) * 256].rearrange(
                    "p (b c) -> p b c", b=4)
                nc.vector.tensor_tensor(h04, tps4[:],
                                        mun4[:].to_broadcast((128, 4, C)),
                                        op=Alu.subtract)
                sq4 = spool.tile([128, 4, C], f32, tag="sq4")
                nc.gpsimd.tensor_tensor(sq4[:], h04, h04, op=Alu.mult)
                ssq4 = spool.tile([128, 4], f32, tag="ssq4")
                nc.vector.tensor_reduce(ssq4[:], sq4[:], Axis.X, Alu.add)
                nc.vector.tensor_scalar(VARS[:, g * 4:(g + 1) * 4], ssq4[:],
                                        1.0 / C, EPS,
                                        op0=Alu.mult, op1=Alu.add)
                nc.scalar.activation(SQV[:, g * 4:(g + 1) * 4],
                                     VARS[:, g * 4:(g + 1) * 4], Act.Sqrt)
                nc.vector.reciprocal(RSTD[:, g * 4:(g + 1) * 4],
                                     SQV[:, g * 4:(g + 1) * 4])
            HNT = bpool.tile([128, L // 2], bf16, name="HNT", tag="HNT")
            for g in range(NCH):
                hnT4 = HNT[:, g * 256:(g + 1) * 256].rearrange(
                    "p (b c) -> p b c", b=4)
                nc.gpsimd.tensor_tensor(
                    hnT4, HN0[:, g * 256:(g + 1) * 256].rearrange(
                        "p (b c) -> p b c", b=4),
                    RSTD[:, g * 4:(g + 1) * 4].to_broadcast((128, 4, C)),
                    op=Alu.mult)
                tb4 = ps.tile([C, 4, 128], bf16, tag="mm")
                for k in range(4):
                    blk = g * 4 + k
                    nc.tensor.transpose(tb4[:, k, :],
                                        HNT[:, blk * C:(blk + 1) * C],
                                        ident)
                nc.vector.tensor_scalar(
                    HN[:, g * CH:(g + 1) * CH],
                    tb4[:].rearrange("p a b -> p (a b)"), 0.0,
                    None, op0=Alu.add)

            # ---- in_proj: xm (DVE bias-add) + z (ACT silu) ----
            for chi in range(NCH):
                sl = slice(chi * CH, (chi + 1) * CH)
                xm_ps = ps.tile([DI, CH], f32, tag="mm")
                z_ps = ps.tile([DI, CH], f32, tag="mm")
                nc.tensor.matmul(xm_ps[:], ip_lhsT[0:C, 0:DI], HN[:, sl],
                                 start=True, stop=True)
                nc.tensor.matmul(z_ps[:], ip_lhsT[0:C, DI:2 * DI], HN[:, sl],
                                 start=True, stop=True)
                nc.vector.tensor_scalar(XM0[:, 4 + chi * CH:4 + (chi + 1) * CH],
                                        xm_ps[:], ip_b0, None, op0=Alu.add)
                nc.scalar.activation(ZS[:, sl], z_ps[:], Act.Silu, bias=ip_b1)

            # ---- causal conv1d on PE (4 diag taps) + silu ----
            for chi in range(NCH):
                sl = slice(chi * CH, (chi + 1) * CH)
                cc = ps.tile([DI, CH], f32, tag="mm")
                for tap in range(4):
                    nc.tensor.matmul(cc[:], cdiag[:, tap * 128:(tap + 1) * 128],
                                     XM0[:, 1 + tap + chi * CH:
                                         1 + tap + chi * CH + CH],
                                     start=(tap == 0), stop=(tap == 3))
                nc.scalar.activation(XC[:, sl], cc[:], Act.Silu, bias=cd_b)

            # ---- x_proj: fused dt matmul; B/C rows; esb exp ----
            for chi in range(NCH):
                sl = slice(chi * CH, (chi + 1) * CH)
                dt_ps = ps.tile([DI, CH], f32, tag="mm")
                nc.tensor.matmul(dt_ps[:], M_dt, XC[:, sl],
                                 start=True, stop=True)
                nc.scalar.activation(ESB[:, sl], dt_ps[:], Act.Exp, bias=dt_b)


            # ---- DT = ln(1+esb) in halves (costs ~2 extra act-table loads
            # but lets U/staging start at the front's midpoint);
            # U = DT*XC; stage U/DT to DRAM for tiled replication ----
            LHf = L // 2
            for hf in range(2):
                hsl = slice(hf * LHf, (hf + 1) * LHf)
                dsl = slice(L + hf * LHf, L + (hf + 1) * LHf)
                nc.scalar.activation(UD[:, dsl], ESB[:, hsl], Act.Ln, bias=1.0)
                nc.vector.tensor_mul(UD[:, hsl], UD[:, dsl], XC[:, hsl])

            # ---- B_tile / C_tile (group-invariant): sel matmul + copy ----
            for chi in range(NCH):
                sl = slice(chi * CH, (chi + 1) * CH)
                bt_ps = ps.tile([DI, CH], f32, tag="mm")
                nc.tensor.matmul(bt_ps[:], W_B, XC[:, sl],
                                 start=True, stop=True)
                nc.scalar.activation(BT[:, sl], bt_ps[:], Act.Copy)
                ct_ps = ps.tile([DI, CH], f32, tag="mm")
                nc.tensor.matmul(ct_ps[:], W_C, XC[:, sl],
                                 start=True, stop=True)
                nc.vector.tensor_scalar(CT[:, sl], ct_ps[:], 0.0, None,
                                        op0=Alu.add)

            # ---- XCD = XC*Dp on Pool (runs during scan phase) ----
            XCD = bpool.tile([DI, L], bf16, name="XCD", tag="ESB")
            for hf in range(2):
                hsl = slice(hf * (L // 2), (hf + 1) * (L // 2))
                nc.gpsimd.tensor_tensor(XCD[:, hsl], XC[:, hsl],
                                        Dp.to_broadcast((DI, L // 2)),
                                        op=Alu.mult)

            # dummy exp: forces the exp act-table load to happen now (ACT
            # idle) instead of being chained onto the first dA exp's waits
            dumex = cpool.tile([128, 1], f32, tag="dumex")
            nc.scalar.activation(dumex[:], BT[:, 0:1], Act.Exp)

            # ---- scan phase: per channel-group g ----
            LH0 = L // 2
            dAe = bpool.tile([DI, LH0], f32, name="dAe", tag="XC")
            ypsA = [psy.tile([DI, CH], f32, name=f"ypsA{ci}", tag="yps")
                    for ci in range(4)]
            for g in range(NG):
                udt = udpool.tile([DI, 2 * L], bf16, tag="udt")
                for j in range(NS):
                    nc.sync.dma_start(
                        udt[j * 16:(j + 1) * 16, :],
                        UD[g * 16:(g + 1) * 16, :])
                for hf in range(2):
                    hsl = slice(hf * LH0, (hf + 1) * LH0)
                    # dbx in-place into the U-half of udt (dead after this)
                    nc.vector.tensor_tensor(udt[:, hsl], udt[:, hsl],
                                            BT[:, hsl], op=Alu.mult)
                    if hf == 0:
                        dA = dAe
                    else:
                        dA = dapool.tile([DI, LH0], f32, tag="dA")
                    nc.scalar.activation(
                        dA[:], udt[:, L + hf * LH0:L + (hf + 1) * LH0],
                        Act.Exp, scale=a_vec[:, g:g + 1])
                    init = 0.0 if hf == 0 else YPs[g][:, LH0 - 1:LH0]
                    nc.vector.tensor_tensor_scan(YPs[g][:, hsl], dA[:],
                                                 udt[:, hsl], init,
                                                 op0=Alu.mult, op1=Alu.add)
                # y partial: YP = H * C_tile (even groups on Pool), split in
                # halves so h0 overlaps the h1 scan and accumulates earlier
                for hf in range(2):
                    hsl2 = slice(hf * LH0, (hf + 1) * LH0)
                    if g != 5 and g != 7:
                        nc.gpsimd.tensor_tensor(YPs[g][:, hsl2], YPs[g][:, hsl2],
                                                CT[:, hsl2], op=Alu.mult)
                    else:
                        nc.vector.tensor_tensor(YPs[g][:, hsl2], YPs[g][:, hsl2],
                                                CT[:, hsl2], op=Alu.mult)
                # incremental y-reduce for chunks 0-3 (PSUM live across phase)
                for ci in range(4):
                    slc = slice(ci * CH, (ci + 1) * CH)
                    nc.tensor.matmul(ypsA[ci][:], Rg[:, g * 128:(g + 1) * 128],
                                     YPs[g][:, slc],
                                     start=(g == 0), stop=(g == NG - 1))

            # ---- y reduce: chunks 0-3 done incrementally; drain + chunks 4-7
            for chi in range(4):
                ysb = spool.tile([DI, CH], bf16, tag="ysb")
                nc.scalar.activation(ysb[:], ypsA[chi][:], Act.Copy)
                nc.sync.dma_start(
                    y_in_t[chi // 2, :, (chi % 2) * CH:(chi % 2 + 1) * CH],
                    ysb[:])
            for chi in range(4, NCH):
                sl = slice(chi * CH, (chi + 1) * CH)
                yps = psy.tile([DI, CH], f32, tag="yps")
                for g in range(NG):
                    nc.tensor.matmul(yps[:], Rg[:, g * 128:(g + 1) * 128],
                                     YPs[g][:, sl],
                                     start=(g == 0), stop=(g == NG - 1))
                ysb = spool.tile([DI, CH], bf16, tag="ysb")
                if chi % 2 == 0:
                    nc.scalar.activation(ysb[:], yps[:], Act.Copy)
                else:
                    nc.vector.tensor_scalar(ysb[:], yps[:], 0.0, None,
                                            op0=Alu.add)
                nc.sync.dma_start(
                    y_in_t[chi // 2, :, (chi % 2) * CH:(chi % 2 + 1) * CH],
                    ysb[:])

            # ---- AllReduce partial y in quarters, pipelined with post/out ----
            # XCD precomputed on Pool (overlaps scan phase)
            YS = bpool.tile([DI, L], bf16, name="YS", tag="HN")
            LQ = L // 4
            for q in range(4):
                qsl = slice(q * LQ, (q + 1) * LQ)
                if sim:
                    nc.sync.dma_start(y_out_t[q], y_in_t[q])
                else:
                    nc.gpsimd.collective_compute(
                        "AllReduce", Alu.add, replica_groups=groups,
                        ins=[y_in_t[q].opt()], outs=[y_out_t[q].opt()])
                nc.sync.dma_start(YSUM[:, qsl], y_out_t[q])
                nc.vector.tensor_add(XCD[:, qsl], YSUM[:, qsl], XCD[:, qsl])
                nc.vector.tensor_mul(YS[:, qsl], XCD[:, qsl], ZS[:, qsl])
                for ci in range(2):
                    chi = q * 2 + ci
                    sl = slice(chi * CH, (chi + 1) * CH)
                    op_ps = psy.tile([C, CH], f32, tag="yps")
                    nc.tensor.matmul(op_ps[:], op_lhsT, YS[:, sl],
                                     start=True, stop=True)
                    outc = spool.tile([C, CH], f32, tag="outc")
                    nc.vector.tensor_tensor(outc[:], op_ps[:], SEQ[:, sl],
                                            op=Alu.add)
                    nc.sync.dma_start(out_d[:, sl], outc[:])

    nc.compile()
    return nc


def _host_precompute(inp):
    import ml_dtypes
    f = lambda k: np.asarray(inp[k], np.float32)
    bf = lambda a: np.ascontiguousarray(a.astype(ml_dtypes.bfloat16))
    w1 = f("conv_w")[:, :, 0, 0]
    wh = f("dwh_w")[:, 0, :, 0]
    ww = f("dww_w")[:, 0, 0, :]
    s_bn = f("bn_g") / np.sqrt(f("bn_v") + EPS)
    taps = [
        w1 * (1.0 + wh[:, 1] + ww[:, 1])[None, :],   # center
        w1 * wh[:, 0][None, :],                       # up
        w1 * wh[:, 2][None, :],                       # down
        w1 * ww[:, 0][None, :],                       # left
        w1 * ww[:, 2][None, :],                       # right
    ]
    cw = np.concatenate([t.T for t in taps], axis=1)
    cw = cw * np.tile(s_bn, 5)[None, :]  # [cin=64, 5*64]
    btot = f("conv_b") + w1 @ (f("dwh_b") + f("dww_b"))
    bn_bias = s_bn * (btot - f("bn_m")) + f("bn_b")
    ipw = f("in_proj_w")
    ip_lhsT = (ipw * f("ln_g")[None, :]).T            # [64, 256]
    ip_bias = ipw @ f("ln_b")                          # [256]
    xpw = f("x_proj_w")                                # [36, 128]
    M_dt = f("dt_proj_w") @ xpw[:DR]                   # [128, 128]
    a_full = -np.exp(np.asarray(inp["A_log"], np.float32))  # [DI, DS]
    cdw = f("convd_w")[:, 0, :]                        # [128, 4]

    per_sigma = []
    for sg in range(2):
        s_lo = sg * NS
        cf32 = np.zeros((128, 32), np.float32)
        cf32[:C, 0] = s_bn
        cf32[:C, 1] = bn_bias
        cf32[:, 2] = ip_bias[:DI]
        cf32[:, 3] = ip_bias[DI:]
        cf32[:, 4] = f("convd_b")
        cf32[:, 5] = f("dt_proj_b")
        cf32[:, 6] = f("Dp")
        # a_vec per group g: a[p] = a_full[16g + p%16, s_lo + p//16]
        p = np.arange(128)
        for g in range(NG):
            cf32[:, 8 + g] = a_full[16 * g + p % 16, s_lo + p // 16]

        cbf = np.zeros((128, 2688), np.float32)
        cbf[:, 0:128] = np.eye(128, dtype=np.float32)
        cbf[:C, 128:448] = cw
        cbf[:C, 448:704] = ip_lhsT
        cbf[:, 704:832] = M_dt.T
        for tap in range(4):
            cbf[:, 832 + tap * 128:832 + (tap + 1) * 128] = np.diag(cdw[:, tap])
        # fused B/C broadcast: W_B[p, :] = xpw_B[s_lo + p//16, :] (stored T)
        for pp in range(128):
            cbf[:, 1344 + pp] = xpw[DR + s_lo + pp // 16]
            cbf[:, 1472 + pp] = xpw[DR + DS + s_lo + pp // 16]
        # Rg: R_g[p, d] = 1 iff d == 16g + p%16
        for g in range(NG):
            for pp in range(128):
                cbf[pp, 1600 + g * 128 + 16 * g + pp % 16] = 1.0
        cbf[:, 2624:2688] = f("out_proj_w").T
        per_sigma.append(dict(cf32=cf32, cbf=bf(cbf)))
    return {}, per_sigma


def _shift_images(xb):
    # 5 pre-shifted copies: ctr, up(reads h-1), dn(h+1), lf(w-1), rt(w+1)
    import ml_dtypes
    out = np.zeros((C, 5, H, W), np.float32)
    out[:, 0] = xb
    out[:, 1, 1:, :] = xb[:, :-1, :]
    out[:, 2, :-1, :] = xb[:, 1:, :]
    out[:, 3, :, 1:] = xb[:, :, :-1]
    out[:, 4, :, :-1] = xb[:, :, 1:]
    return np.ascontiguousarray(
        out.transpose(1, 0, 2, 3).reshape(5, C, L).transpose(1, 0, 2)
        .reshape(C, 5 * L).astype(ml_dtypes.bfloat16))


TRACE = False
LAST_EXEC_NS = None
LAST_TRACE_DIR = None


def kernel(**inputs):
    global LAST_EXEC_NS, LAST_TRACE_DIR
    from concourse.bass_utils import run_bass_kernel_spmd

    if "nc" not in _cached:
        _cached["nc"] = _build_program()
    nc = _cached["nc"]

    common, per_sigma = _host_precompute(inputs)
    x = np.asarray(inputs["x"], np.float32)
    in_maps = []
    for c in range(NCORES):
        b, sg = c // 2, c % 2
        m = dict(common)
        m.update(per_sigma[sg])
        m["ximgs"] = _shift_images(x[b])
        in_maps.append(m)

    kw = {}
    if TRACE:
        import tempfile
        LAST_TRACE_DIR = tempfile.mkdtemp(prefix="bass_trace_")
        kw = dict(trace=True, tmpdir=LAST_TRACE_DIR)
    r = run_bass_kernel_spmd(nc, in_maps, list(range(NCORES)), **kw)
    if r.exec_time_ns is not None:
        LAST_EXEC_NS = r.exec_time_ns
    res = r.results
    out = np.empty((B, C, H, W), np.float32)
    for b in range(B):
        out[b] = np.asarray(res[2 * b]["out_f"], np.float32).reshape(C, H, W)
    return out


# revision 51
# speedup vs baseline: 1.0895x; 1.0141x over previous
"""Trainium2 Bass kernel v2 for nn_DecoderBlock_Mamba.

Sharding: 8 cores = (batch b in 0..3) x (state-half sigma in {0,1}).
Scan uses a tiled partition layout: partition p = j*16 + i holds state
(s_lo + j) and channel-group offset i; 8 channel-groups g cover d = 16g + i.
This makes the B/C broadcasts group-invariant (built once) and both dbx / y
multiplies all-SBUF-bf16 (2x DVE fast path). U/DT are staged to DRAM and
replicated into the tiled layout by 8 DMAs per group (DMA engines are idle).

Self-contained: hardcodes all shapes; no sibling imports.
"""
import numpy as np

C = 64
DI = 128
DS = 16
DR = 4
B = 4
H = 64
W = 64
L = H * W
NS = 8            # states per core
NG = 8            # channel groups (of 16) per core
NCORES = 8
NCH = 8           # L chunks of 512
CH = 512
EPS = 1e-5

_cached = {}


def _build_program(sim=False, phases=3):
    import concourse.bass as bass
    import concourse.bacc as bacc
    import concourse.mybir as mybir
    import concourse.tile as tile

    dt = mybir.dt
    f32 = dt.float32
    bf16 = dt.bfloat16
    Act = mybir.ActivationFunctionType
    Alu = mybir.AluOpType
    Axis = mybir.AxisListType

    nc = bacc.Bacc(None, target_bir_lowering=False)

    def din(name, shape, dtype=f32):
        return nc.dram_tensor(name, shape, dtype, kind="ExternalInput")

    ximgs_d = din("ximgs", [C, 5 * L], bf16)
    cf32_d = din("cf32", [128, 32])
    cbf_d = din("cbf", [128, 2688], bf16)

    out_d = nc.dram_tensor("out_f", [C, L], f32, kind="ExternalOutput")

    groups = [[0, 1], [2, 3], [4, 5], [6, 7]]

    with tile.TileContext(nc) as tc:
        with (
            tc.tile_pool(name="dram", bufs=1, space="DRAM") as dpool,
            tc.tile_pool(name="const", bufs=1) as cpool,
            tc.tile_pool(name="big", bufs=1) as bpool,
            tc.tile_pool(name="sm", bufs=2) as spool,
            tc.tile_pool(name="ud", bufs=2) as udpool,
            tc.tile_pool(name="da", bufs=1) as dapool,
            tc.tile_pool(name="ps", bufs=4, space="PSUM") as ps,
            tc.tile_pool(name="psy", bufs=4, space="PSUM") as psy,
        ):
            # ---- constants (packed: 2 DMAs) ----
            cf = cpool.tile([128, 32], f32)
            cb = cpool.tile([128, 2688], bf16)
            nc.sync.dma_start(cf[:], cf32_d[:])
            nc.sync.dma_start(cb[:], cbf_d[:])
            bn_s = cf[0:C, 0:1]
            bn_b = cf[0:C, 1:2]
            ip_b0 = cf[:, 2:3]
            ip_b1 = cf[:, 3:4]
            cd_b = cf[:, 4:5]
            dt_b = cf[:, 5:6]
            Dp = cf[:, 6:7]
            a_vec = cf[:, 8:16]          # per-group a scale [128, 8]

            ident = cb[:, 0:128]
            cw = cb[0:C, 128:448]
            ip_lhsT = cb[0:C, 448:704]
            M_dt = cb[:, 704:832]
            cdiag = cb[:, 832:1344]      # 4 diag taps [128, 4*128]
            W_B = cb[:, 1344:1472]       # fused B broadcast [128,128]
            W_C = cb[:, 1472:1600]       # fused C broadcast
            Rg = cb[:, 1600:2624]        # 8 x [128,128] reduce mats
            op_lhsT = cb[:, 2624:2688]

            # ---- persistent activations ----
            SEQ = bpool.tile([C, L], bf16)            # BN+ReLU out (residual)
            HN = bpool.tile([C, L], bf16)             # LN-normalized
            XM0 = bpool.tile([DI, L + 4], bf16, name="XM0", tag="YP5")       # conv1d in, data @ col 4
            ZS = bpool.tile([DI, L], bf16)            # silu(z)
            XC = bpool.tile([DI, L], bf16)
            ESB = bpool.tile([DI, L], bf16, tag="ESB")
            UD = bpool.tile([DI, 2 * L], bf16, name="UD", tag="HN")
            BT = bpool.tile([DI, L], bf16, name="BT")      # B_tile (j slow)
            CT = bpool.tile([DI, L], bf16, name="CT")      # C_tile
            YPs = [bpool.tile([DI, L], bf16, name=f"YP{g}", tag=f"YP{g}")
                   for g in range(NG)]
            YSUM = bpool.tile([DI, L], bf16, name="YSUM", tag="YSUM")

            y_in_t = dpool.tile([4, DI, L // 4], bf16, tag="yin")
            y_out_t = dpool.tile([4, DI, L // 4], bf16, tag="yout")

            # Prime ACT's vector clock on the const DMAs
            warm = cpool.tile([128, 1], f32, tag="warm")
            nc.scalar.activation(warm[:], cf[:, 0:1], Act.Copy)
            warm2 = cpool.tile([128, 1], bf16, tag="warm2")
            nc.scalar.activation(warm2[:], cb[:, 0:1], Act.Copy)
            nc.vector.tensor_scalar_mul(XM0[:, 0:4], cf[:, 0:4], 0.0)

            IMGS = [bpool.tile([C, L], bf16, name=f"img{t}", tag=f"YP{t}")
                    for t in range(5)]
            for t in range(5):
                nc.sync.dma_start(IMGS[t][:], ximgs_d[:, t * L:(t + 1) * L])

            # ---- front conv: 5 accumulating taps + BN + ReLU ----
            for chi in range(NCH):
                sl = slice(chi * CH, (chi + 1) * CH)
                pc = ps.tile([C, CH], f32, tag="mm")
                for tap in range(5):
                    nc.tensor.matmul(pc[:], cw[:, tap * C:(tap + 1) * C],
                                     IMGS[tap][:, sl],
                                     start=(tap == 0), stop=(tap == 4))
                nc.scalar.activation(SEQ[:, sl], pc[:],
                                     Act.Relu, bias=bn_b)

            # ---- LayerNorm over channels, batched 4 blocks per op ----
            HN0 = bpool.tile([128, L // 2], bf16, name="HN0", tag="HN0")
            VARS = spool.tile([128, 32], f32, tag="VARS")
            SQV = spool.tile([128, 32], f32, tag="SQV")
            RSTD = spool.tile([128, 32], f32, tag="RSTD")
            for g in range(NCH):
                tps4 = ps.tile([128, 4, C], bf16, tag="mm")
                for k in range(4):
                    blk = g * 4 + k
                    nc.tensor.transpose(tps4[:, k, :],
                                        SEQ[:, blk * 128:(blk + 1) * 128],
                                        ident[0:C, 0:C])
                mu4 = spool.tile([128, 4], f32, tag="mu4")
                nc.vector.tensor_reduce(mu4[:], tps4[:], Axis.X, Alu.add)
                mun4 = spool.tile([128, 4], f32, tag="mun4")
                nc.vector.tensor_scalar_mul(mun4[:], mu4[:], 1.0 / C)
                h04 = HN0[:, g * 256:(g + 1) * 256].rearrange(
                    "p (b c) -> p b c", b=4)
                nc.vector.tensor_tensor(h04, tps4[:],
                                        mun4[:].to_broadcast((128, 4, C)),
                                        op=Alu.subtract)
                sq4 = spool.tile([128, 4, C], f32, tag="sq4")
                nc.gpsimd.tensor_tensor(sq4[:], h04, h04, op=Alu.mult)
                ssq4 = spool.tile([128, 4], f32, tag="ssq4")
                nc.vector.tensor_reduce(ssq4[:], sq4[:], Axis.X, Alu.add)
                nc.vector.tensor_scalar(VARS[:, g * 4:(g + 1) * 4], ssq4[:],
                                        1.0 / C, EPS,
                                        op0=Alu.mult, op1=Alu.add)
                nc.scalar.activation(SQV[:, g * 4:(g + 1) * 4],
                                     VARS[:, g * 4:(g + 1) * 4], Act.Sqrt)
                nc.vector.reciprocal(RSTD[:, g * 4:(g + 1) * 4],
                                     SQV[:, g * 4:(g + 1) * 4])
            HNT = bpool.tile([128, L // 2], bf16, name="HNT", tag="HNT")
            for g in range(NCH):
                hnT4 = HNT[:, g * 256:(g + 1) * 256].rearrange(
                    "p (b c) -> p b c", b=4)
                nc.gpsimd.tensor_tensor(
                    hnT4, HN0[:, g * 256:(g + 1) * 256].rearrange(
                        "p (b c) -> p b c", b=4),
                    RSTD[:, g * 4:(g + 1) * 4].to_broadcast((128, 4, C)),
                    op=Alu.mult)
                tb4 = ps.tile([C, 4, 128], bf16, tag="mm")
                for k in range(4):
                    blk = g * 4 + k
                    nc.tensor.transpose(tb4[:, k, :],
                                        HNT[:, blk * C:(blk + 1) * C],
                                        ident)
                nc.vector.tensor_scalar(
                    HN[:, g * CH:(g + 1) * CH],
                    tb4[:].rearrange("p a b -> p (a b)"), 0.0,
                    None, op0=Alu.add)

            # ---- in_proj: xm (DVE bias-add) + z (ACT silu) ----
            for chi in range(NCH):
                sl = slice(chi * CH, (chi + 1) * CH)
                xm_ps = ps.tile([DI, CH], f32, tag="mm")
                z_ps = ps.tile([DI, CH], f32, tag="mm")
                nc.tensor.matmul(xm_ps[:], ip_lhsT[0:C, 0:DI], HN[:, sl],
                                 start=True, stop=True)
                nc.tensor.matmul(z_ps[:], ip_lhsT[0:C, DI:2 * DI], HN[:, sl],
                                 start=True, stop=True)
                nc.vector.tensor_scalar(XM0[:, 4 + chi * CH:4 + (chi + 1) * CH],
                                        xm_ps[:], ip_b0, None, op0=Alu.add)
                nc.scalar.activation(ZS[:, sl], z_ps[:], Act.Silu, bias=ip_b1)

            # ---- causal conv1d on PE (4 diag taps) + silu ----
            for chi in range(NCH):
                sl = slice(chi * CH, (chi + 1) * CH)
                cc = ps.tile([DI, CH], f32, tag="mm")
                for tap in range(4):
                    nc.tensor.matmul(cc[:], cdiag[:, tap * 128:(tap + 1) * 128],
                                     XM0[:, 1 + tap + chi * CH:
                                         1 + tap + chi * CH + CH],
                                     start=(tap == 0), stop=(tap == 3))
                nc.scalar.activation(XC[:, sl], cc[:], Act.Silu, bias=cd_b)

            # ---- x_proj: fused dt matmul; B/C rows; esb exp ----
            for chi in range(NCH):
                sl = slice(chi * CH, (chi + 1) * CH)
                dt_ps = ps.tile([DI, CH], f32, tag="mm")
                nc.tensor.matmul(dt_ps[:], M_dt, XC[:, sl],
                                 start=True, stop=True)
                nc.scalar.activation(ESB[:, sl], dt_ps[:], Act.Exp, bias=dt_b)


            # ---- DT = ln(1+esb) in halves (costs ~2 extra act-table loads
            # but lets U/staging start at the front's midpoint);
            # U = DT*XC; stage U/DT to DRAM for tiled replication ----
            LHf = L // 2
            for hf in range(2):
                hsl = slice(hf * LHf, (hf + 1) * LHf)
                dsl = slice(L + hf * LHf, L + (hf + 1) * LHf)
                nc.scalar.activation(UD[:, dsl], ESB[:, hsl], Act.Ln, bias=1.0)
                nc.vector.tensor_mul(UD[:, hsl], UD[:, dsl], XC[:, hsl])

            # ---- B_tile / C_tile (group-invariant): sel matmul + copy ----
            for chi in range(NCH):
                sl = slice(chi * CH, (chi + 1) * CH)
                bt_ps = ps.tile([DI, CH], f32, tag="mm")
                nc.tensor.matmul(bt_ps[:], W_B, XC[:, sl],
                                 start=True, stop=True)
                nc.scalar.activation(BT[:, sl], bt_ps[:], Act.Copy)
                ct_ps = ps.tile([DI, CH], f32, tag="mm")
                nc.tensor.matmul(ct_ps[:], W_C, XC[:, sl],
                                 start=True, stop=True)
                nc.vector.tensor_scalar(CT[:, sl], ct_ps[:], 0.0, None,
                                        op0=Alu.add)

            # ---- XCD = XC*Dp on Pool (runs during scan phase) ----
            XCD = bpool.tile([DI, L], bf16, name="XCD", tag="ESB")
            for hf in range(2):
                hsl = slice(hf * (L // 2), (hf + 1) * (L // 2))
                nc.gpsimd.tensor_tensor(XCD[:, hsl], XC[:, hsl],
                                        Dp.to_broadcast((DI, L // 2)),
                                        op=Alu.mult)

            # dummy exp: forces the exp act-table load to happen now (ACT
            # idle) instead of being chained onto the first dA exp's waits
            dumex = cpool.tile([128, 1], f32, tag="dumex")
            nc.scalar.activation(dumex[:], BT[:, 0:1], Act.Exp)

            # ---- scan phase: per channel-group g ----
            LH0 = L // 2
            dAe = bpool.tile([DI, LH0], f32, name="dAe", tag="XC")
            ypsA = [psy.tile([DI, CH], f32, name=f"ypsA{ci}", tag="yps")
                    for ci in range(4)]
            for g in range(NG):
                udt = udpool.tile([DI, 2 * L], bf16, tag="udt")
                for j in range(NS):
                    nc.sync.dma_start(
                        udt[j * 16:(j + 1) * 16, :],
                        UD[g * 16:(g + 1) * 16, :])
                for hf in range(2):
                    hsl = slice(hf * LH0, (hf + 1) * LH0)
                    # dbx in-place into the U-half of udt (dead after this)
                    nc.vector.tensor_tensor(udt[:, hsl], udt[:, hsl],
                                            BT[:, hsl], op=Alu.mult)
                    if hf == 0:
                        dA = dAe
                    else:
                        dA = dapool.tile([DI, LH0], f32, tag="dA")
                    nc.scalar.activation(
                        dA[:], udt[:, L + hf * LH0:L + (hf + 1) * LH0],
                        Act.Exp, scale=a_vec[:, g:g + 1])
                    init = 0.0 if hf == 0 else YPs[g][:, LH0 - 1:LH0]
                    nc.vector.tensor_tensor_scan(YPs[g][:, hsl], dA[:],
                                                 udt[:, hsl], init,
                                                 op0=Alu.mult, op1=Alu.add)
                    if g == NG - 1 and hf == 0:
                        # last group's h0 y-mul goes into udt's dead dbx
                        # region: no in-place init conflict, so it runs
                        # during the h1 scan and the drain starts earlier
                        nc.vector.tensor_tensor(udt[:, hsl], YPs[g][:, hsl],
                                                CT[:, hsl], op=Alu.mult)
                # y partial: YP = H * C_tile (even groups on Pool), split in
                # halves so h0 overlaps the h1 scan and accumulates earlier
                for hf in range(2):
                    hsl2 = slice(hf * LH0, (hf + 1) * LH0)
                    if g == NG - 1 and hf == 0:
                        continue          # already done into udt above
                    if g != 5 and g != 7:
                        nc.gpsimd.tensor_tensor(YPs[g][:, hsl2], YPs[g][:, hsl2],
                                                CT[:, hsl2], op=Alu.mult)
                    else:
                        nc.vector.tensor_tensor(YPs[g][:, hsl2], YPs[g][:, hsl2],
                                                CT[:, hsl2], op=Alu.mult)
                # incremental y-reduce for chunks 0-3 (PSUM live across phase)
                for ci in range(4):
                    slc = slice(ci * CH, (ci + 1) * CH)
                    rhs = udt[:, slc] if g == NG - 1 else YPs[g][:, slc]
                    nc.tensor.matmul(ypsA[ci][:], Rg[:, g * 128:(g + 1) * 128],
                                     rhs, start=(g == 0), stop=(g == NG - 1))

            # ---- y reduce: chunks 0-3 done incrementally; drain + chunks 4-7
            for chi in range(4):
                ysb = spool.tile([DI, CH], bf16, tag="ysb")
                nc.scalar.activation(ysb[:], ypsA[chi][:], Act.Copy)
                nc.sync.dma_start(
                    y_in_t[chi // 2, :, (chi % 2) * CH:(chi % 2 + 1) * CH],
                    ysb[:])
            for chi in range(4, NCH):
                sl = slice(chi * CH, (chi + 1) * CH)
                yps = psy.tile([DI, CH], f32, tag="yps")
                for g in range(NG):
                    nc.tensor.matmul(yps[:], Rg[:, g * 128:(g + 1) * 128],
                                     YPs[g][:, sl],
                                     start=(g == 0), stop=(g == NG - 1))
                ysb = spool.tile([DI, CH], bf16, tag="ysb")
                if chi % 2 == 0:
                    nc.scalar.activation(ysb[:], yps[:], Act.Copy)
                else:
                    nc.vector.tensor_scalar(ysb[:], yps[:], 0.0, None,
                                            op0=Alu.add)
                nc.sync.dma_start(
                    y_in_t[chi // 2, :, (chi % 2) * CH:(chi % 2 + 1) * CH],
                    ysb[:])

            # ---- AllReduce partial y in quarters, pipelined with post/out ----
            # XCD precomputed on Pool (overlaps scan phase)
            YS = bpool.tile([DI, L], bf16, name="YS", tag="HN")
            LQ = L // 4
            for q in range(4):
                qsl = slice(q * LQ, (q + 1) * LQ)
                if sim:
                    nc.sync.dma_start(y_out_t[q], y_in_t[q])
                else:
                    nc.gpsimd.collective_compute(
                        "AllReduce", Alu.add, replica_groups=groups,
                        ins=[y_in_t[q].opt()], outs=[y_out_t[q].opt()])
                nc.sync.dma_start(YSUM[:, qsl], y_out_t[q])
                nc.vector.tensor_add(XCD[:, qsl], YSUM[:, qsl], XCD[:, qsl])
                nc.vector.tensor_mul(YS[:, qsl], XCD[:, qsl], ZS[:, qsl])
                for ci in range(2):
                    chi = q * 2 + ci
                    sl = slice(chi * CH, (chi + 1) * CH)
                    op_ps = psy.tile([C, CH], f32, tag="yps")
                    nc.tensor.matmul(op_ps[:], op_lhsT, YS[:, sl],
                                     start=True, stop=True)
                    outc = spool.tile([C, CH], f32, tag="outc")
                    nc.vector.tensor_tensor(outc[:], op_ps[:], SEQ[:, sl],
                                            op=Alu.add)
                    nc.sync.dma_start(out_d[:, sl], outc[:])

    nc.compile()
    return nc


def _host_precompute(inp):
    import ml_dtypes
    f = lambda k: np.asarray(inp[k], np.float32)
    bf = lambda a: np.ascontiguousarray(a.astype(ml_dtypes.bfloat16))
    w1 = f("conv_w")[:, :, 0, 0]
    wh = f("dwh_w")[:, 0, :, 0]
    ww = f("dww_w")[:, 0, 0, :]
    s_bn = f("bn_g") / np.sqrt(f("bn_v") + EPS)
    taps = [
        w1 * (1.0 + wh[:, 1] + ww[:, 1])[None, :],   # center
        w1 * wh[:, 0][None, :],                       # up
        w1 * wh[:, 2][None, :],                       # down
        w1 * ww[:, 0][None, :],                       # left
        w1 * ww[:, 2][None, :],                       # right
    ]
    cw = np.concatenate([t.T for t in taps], axis=1)
    cw = cw * np.tile(s_bn, 5)[None, :]  # [cin=64, 5*64]
    btot = f("conv_b") + w1 @ (f("dwh_b") + f("dww_b"))
    bn_bias = s_bn * (btot - f("bn_m")) + f("bn_b")
    ipw = f("in_proj_w")
    ip_lhsT = (ipw * f("ln_g")[None, :]).T            # [64, 256]
    ip_bias = ipw @ f("ln_b")                          # [256]
    xpw = f("x_proj_w")                                # [36, 128]
    M_dt = f("dt_proj_w") @ xpw[:DR]                   # [128, 128]
    a_full = -np.exp(np.asarray(inp["A_log"], np.float32))  # [DI, DS]
    cdw = f("convd_w")[:, 0, :]                        # [128, 4]

    per_sigma = []
    for sg in range(2):
        s_lo = sg * NS
        cf32 = np.zeros((128, 32), np.float32)
        cf32[:C, 0] = s_bn
        cf32[:C, 1] = bn_bias
        cf32[:, 2] = ip_bias[:DI]
        cf32[:, 3] = ip_bias[DI:]
        cf32[:, 4] = f("convd_b")
        cf32[:, 5] = f("dt_proj_b")
        cf32[:, 6] = f("Dp")
        # a_vec per group g: a[p] = a_full[16g + p%16, s_lo + p//16]
        p = np.arange(128)
        for g in range(NG):
            cf32[:, 8 + g] = a_full[16 * g + p % 16, s_lo + p // 16]

        cbf = np.zeros((128, 2688), np.float32)
        cbf[:, 0:128] = np.eye(128, dtype=np.float32)
        cbf[:C, 128:448] = cw
        cbf[:C, 448:704] = ip_lhsT
        cbf[:, 704:832] = M_dt.T
        for tap in range(4):
            cbf[:, 832 + tap * 128:832 + (tap + 1) * 128] = np.diag(cdw[:, tap])
        # fused B/C broadcast: W_B[p, :] = xpw_B[s_lo + p//16, :] (stored T)
        for pp in range(128):
            cbf[:, 1344 + pp] = xpw[DR + s_lo + pp // 16]
            cbf[:, 1472 + pp] = xpw[DR + DS + s_lo + pp // 16]
        # Rg: R_g[p, d] = 1 iff d == 16g + p%16
        for g in range(NG):
            for pp in range(128):
                cbf[pp, 1600 + g * 128 + 16 * g + pp % 16] = 1.0
        cbf[:, 2624:2688] = f("out_proj_w").T
        per_sigma.append(dict(cf32=cf32, cbf=bf(cbf)))
    return {}, per_sigma


def _shift_images(xb):
    # 5 pre-shifted copies: ctr, up(reads h-1), dn(h+1), lf(w-1), rt(w+1)
    import ml_dtypes
    out = np.zeros((C, 5, H, W), np.float32)
    out[:, 0] = xb
    out[:, 1, 1:, :] = xb[:, :-1, :]
    out[:, 2, :-1, :] = xb[:, 1:, :]
    out[:, 3, :, 1:] = xb[:, :, :-1]
    out[:, 4, :, :-1] = xb[:, :, 1:]
    return np.ascontiguousarray(
        out.transpose(1, 0, 2, 3).reshape(5, C, L).transpose(1, 0, 2)
        .reshape(C, 5 * L).astype(ml_dtypes.bfloat16))


TRACE = False
LAST_EXEC_NS = None
LAST_TRACE_DIR = None


def kernel(**inputs):
    global LAST_EXEC_NS, LAST_TRACE_DIR
    from concourse.bass_utils import run_bass_kernel_spmd

    if "nc" not in _cached:
        _cached["nc"] = _build_program()
    nc = _cached["nc"]

    common, per_sigma = _host_precompute(inputs)
    x = np.asarray(inputs["x"], np.float32)
    in_maps = []
    for c in range(NCORES):
        b, sg = c // 2, c % 2
        m = dict(common)
        m.update(per_sigma[sg])
        m["ximgs"] = _shift_images(x[b])
        in_maps.append(m)

    kw = {}
    if TRACE:
        import tempfile
        LAST_TRACE_DIR = tempfile.mkdtemp(prefix="bass_trace_")
        kw = dict(trace=True, tmpdir=LAST_TRACE_DIR)
    r = run_bass_kernel_spmd(nc, in_maps, list(range(NCORES)), **kw)
    if r.exec_time_ns is not None:
        LAST_EXEC_NS = r.exec_time_ns
    res = r.results
    out = np.empty((B, C, H, W), np.float32)
    for b in range(B):
        out[b] = np.asarray(res[2 * b]["out_f"], np.float32).reshape(C, H, W)
    return out


# revision 56
# speedup vs baseline: 1.0937x; 1.0039x over previous
"""Trainium2 Bass kernel v2 for nn_DecoderBlock_Mamba.

Sharding: 8 cores = (batch b in 0..3) x (state-half sigma in {0,1}).
Scan uses a tiled partition layout: partition p = j*16 + i holds state
(s_lo + j) and channel-group offset i; 8 channel-groups g cover d = 16g + i.
This makes the B/C broadcasts group-invariant (built once) and both dbx / y
multiplies all-SBUF-bf16 (2x DVE fast path). U/DT are staged to DRAM and
replicated into the tiled layout by 8 DMAs per group (DMA engines are idle).

Self-contained: hardcodes all shapes; no sibling imports.
"""
import numpy as np

C = 64
DI = 128
DS = 16
DR = 4
B = 4
H = 64
W = 64
L = H * W
NS = 8            # states per core
NG = 8            # channel groups (of 16) per core
NCORES = 8
NCH = 8           # L chunks of 512
CH = 512
EPS = 1e-5

_cached = {}


def _build_program(sim=False, phases=3):
    import concourse.bass as bass
    import concourse.bacc as bacc
    import concourse.mybir as mybir
    import concourse.tile as tile

    dt = mybir.dt
    f32 = dt.float32
    bf16 = dt.bfloat16
    Act = mybir.ActivationFunctionType
    Alu = mybir.AluOpType
    Axis = mybir.AxisListType

    nc = bacc.Bacc(None, target_bir_lowering=False)

    def din(name, shape, dtype=f32):
        return nc.dram_tensor(name, shape, dtype, kind="ExternalInput")

    ximgs_d = din("ximgs", [C, 5 * L], bf16)
    cf32_d = din("cf32", [128, 32])
    cbf_d = din("cbf", [128, 2688], bf16)

    out_d = nc.dram_tensor("out_f", [C, L], f32, kind="ExternalOutput")

    groups = [[0, 1], [2, 3], [4, 5], [6, 7]]

    with tile.TileContext(nc) as tc:
        with (
            tc.tile_pool(name="dram", bufs=1, space="DRAM") as dpool,
            tc.tile_pool(name="const", bufs=1) as cpool,
            tc.tile_pool(name="big", bufs=1) as bpool,
            tc.tile_pool(name="sm", bufs=2) as spool,
            tc.tile_pool(name="ud", bufs=2) as udpool,
            tc.tile_pool(name="da", bufs=1) as dapool,
            tc.tile_pool(name="ps", bufs=4, space="PSUM") as ps,
            tc.tile_pool(name="psy", bufs=4, space="PSUM") as psy,
        ):
            # ---- constants (packed: 2 DMAs) ----
            cf = cpool.tile([128, 32], f32)
            cb = cpool.tile([128, 2688], bf16)
            nc.sync.dma_start(cf[:], cf32_d[:])
            nc.sync.dma_start(cb[:], cbf_d[:])
            bn_s = cf[0:C, 0:1]
            bn_b = cf[0:C, 1:2]
            ip_b0 = cf[:, 2:3]
            ip_b1 = cf[:, 3:4]
            cd_b = cf[:, 4:5]
            dt_b = cf[:, 5:6]
            Dp = cf[:, 6:7]
            a_vec = cf[:, 8:16]          # per-group a scale [128, 8]

            ident = cb[:, 0:128]
            cw = cb[0:C, 128:448]
            ip_lhsT = cb[0:C, 448:704]
            M_dt = cb[:, 704:832]
            cdiag = cb[:, 832:1344]      # 4 diag taps [128, 4*128]
            W_B = cb[:, 1344:1472]       # fused B broadcast [128,128]
            W_C = cb[:, 1472:1600]       # fused C broadcast
            Rg = cb[:, 1600:2624]        # 8 x [128,128] reduce mats
            op_lhsT = cb[:, 2624:2688]

            # ---- persistent activations ----
            SEQ = bpool.tile([C, L], bf16)            # BN+ReLU out (residual)
            HN = bpool.tile([C, L], bf16)             # LN-normalized
            XM0 = bpool.tile([DI, L + 4], bf16, name="XM0", tag="YP5")       # conv1d in, data @ col 4
            ZS = bpool.tile([DI, L], bf16)            # silu(z)
            XC = bpool.tile([DI, L], bf16)
            ESB = bpool.tile([DI, L], bf16, tag="ESB")
            UD = bpool.tile([DI, 2 * L], bf16, name="UD", tag="HN")
            BT = bpool.tile([DI, L], bf16, name="BT")      # B_tile (j slow)
            CT = bpool.tile([DI, L], bf16, name="CT")      # C_tile
            YPs = [bpool.tile([DI, L], bf16, name=f"YP{g}", tag=f"YP{g}")
                   for g in range(NG)]
            YSUM = bpool.tile([DI, L], bf16, name="YSUM", tag="YSUM")

            y_in_t = dpool.tile([4, DI, L // 4], bf16, tag="yin")
            y_out_t = dpool.tile([4, DI, L // 4], bf16, tag="yout")

            # Prime ACT's vector clock on the const DMAs
            warm = cpool.tile([128, 1], f32, tag="warm")
            nc.scalar.activation(warm[:], cf[:, 0:1], Act.Copy)
            warm2 = cpool.tile([128, 1], bf16, tag="warm2")
            nc.scalar.activation(warm2[:], cb[:, 0:1], Act.Copy)
            nc.vector.tensor_scalar_mul(XM0[:, 0:4], cf[:, 0:4], 0.0)

            IMGS = [bpool.tile([C, L], bf16, name=f"img{t}", tag=f"YP{t}")
                    for t in range(5)]
            for t in range(5):
                nc.sync.dma_start(IMGS[t][:], ximgs_d[:, t * L:(t + 1) * L])

            # ---- front conv: 5 accumulating taps + BN + ReLU ----
            for chi in range(NCH):
                sl = slice(chi * CH, (chi + 1) * CH)
                pc = ps.tile([C, CH], f32, tag="mm")
                for tap in range(5):
                    nc.tensor.matmul(pc[:], cw[:, tap * C:(tap + 1) * C],
                                     IMGS[tap][:, sl],
                                     start=(tap == 0), stop=(tap == 4))
                nc.scalar.activation(SEQ[:, sl], pc[:],
                                     Act.Relu, bias=bn_b)

            # ---- LayerNorm over channels, batched 4 blocks per op ----
            HN0 = bpool.tile([128, L // 2], bf16, name="HN0", tag="HN0")
            VARS = spool.tile([128, 32], f32, tag="VARS")
            SQV = spool.tile([128, 32], f32, tag="SQV")
            RSTD = spool.tile([128, 32], f32, tag="RSTD")
            for g in range(NCH):
                tps4 = ps.tile([128, 4, C], bf16, tag="mm")
                for k in range(4):
                    blk = g * 4 + k
                    nc.tensor.transpose(tps4[:, k, :],
                                        SEQ[:, blk * 128:(blk + 1) * 128],
                                        ident[0:C, 0:C])
                mu4 = spool.tile([128, 4], f32, tag="mu4")
                nc.vector.tensor_reduce(mu4[:], tps4[:], Axis.X, Alu.add)
                mun4 = spool.tile([128, 4], f32, tag="mun4")
                nc.vector.tensor_scalar_mul(mun4[:], mu4[:], 1.0 / C)
                h04 = HN0[:, g * 256:(g + 1) * 256].rearrange(
                    "p (b c) -> p b c", b=4)
                nc.vector.tensor_tensor(h04, tps4[:],
                                        mun4[:].to_broadcast((128, 4, C)),
                                        op=Alu.subtract)
                sq4 = spool.tile([128, 4, C], f32, tag="sq4")
                nc.gpsimd.tensor_tensor(sq4[:], h04, h04, op=Alu.mult)
                ssq4 = spool.tile([128, 4], f32, tag="ssq4")
                nc.vector.tensor_reduce(ssq4[:], sq4[:], Axis.X, Alu.add)
                nc.vector.tensor_scalar(VARS[:, g * 4:(g + 1) * 4], ssq4[:],
                                        1.0 / C, EPS,
                                        op0=Alu.mult, op1=Alu.add)
                nc.scalar.activation(SQV[:, g * 4:(g + 1) * 4],
                                     VARS[:, g * 4:(g + 1) * 4], Act.Sqrt)
                nc.vector.reciprocal(RSTD[:, g * 4:(g + 1) * 4],
                                     SQV[:, g * 4:(g + 1) * 4])
            HNT = bpool.tile([128, L // 2], bf16, name="HNT", tag="HNT")
            for g in range(NCH):
                hnT4 = HNT[:, g * 256:(g + 1) * 256].rearrange(
                    "p (b c) -> p b c", b=4)
                nc.gpsimd.tensor_tensor(
                    hnT4, HN0[:, g * 256:(g + 1) * 256].rearrange(
                        "p (b c) -> p b c", b=4),
                    RSTD[:, g * 4:(g + 1) * 4].to_broadcast((128, 4, C)),
                    op=Alu.mult)
                tb4 = ps.tile([C, 4, 128], bf16, tag="mm")
                for k in range(4):
                    blk = g * 4 + k
                    nc.tensor.transpose(tb4[:, k, :],
                                        HNT[:, blk * C:(blk + 1) * C],
                                        ident)
                nc.vector.tensor_scalar(
                    HN[:, g * CH:(g + 1) * CH],
                    tb4[:].rearrange("p a b -> p (a b)"), 0.0,
                    None, op0=Alu.add)

            # ---- in_proj: xm (DVE bias-add) + z (ACT silu) ----
            for chi in range(NCH):
                sl = slice(chi * CH, (chi + 1) * CH)
                xm_ps = ps.tile([DI, CH], f32, tag="mm")
                z_ps = ps.tile([DI, CH], f32, tag="mm")
                nc.tensor.matmul(xm_ps[:], ip_lhsT[0:C, 0:DI], HN[:, sl],
                                 start=True, stop=True)
                nc.tensor.matmul(z_ps[:], ip_lhsT[0:C, DI:2 * DI], HN[:, sl],
                                 start=True, stop=True)
                nc.vector.tensor_scalar(XM0[:, 4 + chi * CH:4 + (chi + 1) * CH],
                                        xm_ps[:], ip_b0, None, op0=Alu.add)
                nc.scalar.activation(ZS[:, sl], z_ps[:], Act.Silu, bias=ip_b1)

            # ---- causal conv1d on PE (4 diag taps) + silu ----
            for chi in range(NCH):
                sl = slice(chi * CH, (chi + 1) * CH)
                cc = ps.tile([DI, CH], f32, tag="mm")
                for tap in range(4):
                    nc.tensor.matmul(cc[:], cdiag[:, tap * 128:(tap + 1) * 128],
                                     XM0[:, 1 + tap + chi * CH:
                                         1 + tap + chi * CH + CH],
                                     start=(tap == 0), stop=(tap == 3))
                nc.scalar.activation(XC[:, sl], cc[:], Act.Silu, bias=cd_b)

            # ---- x_proj: fused dt matmul; B/C rows; esb exp ----
            for chi in range(NCH):
                sl = slice(chi * CH, (chi + 1) * CH)
                dt_ps = ps.tile([DI, CH], f32, tag="mm")
                nc.tensor.matmul(dt_ps[:], M_dt, XC[:, sl],
                                 start=True, stop=True)
                nc.scalar.activation(ESB[:, sl], dt_ps[:], Act.Exp, bias=dt_b)


            # ---- DT = ln(1+esb) in halves (costs ~2 extra act-table loads
            # but lets U/staging start at the front's midpoint);
            # U = DT*XC; stage U/DT to DRAM for tiled replication ----
            LHf = L // 2
            for hf in range(2):
                hsl = slice(hf * LHf, (hf + 1) * LHf)
                dsl = slice(L + hf * LHf, L + (hf + 1) * LHf)
                nc.scalar.activation(UD[:, dsl], ESB[:, hsl], Act.Ln, bias=1.0)
                nc.vector.tensor_mul(UD[:, hsl], UD[:, dsl], XC[:, hsl])

            # ---- B_tile / C_tile (group-invariant): sel matmul + copy ----
            for chi in range(NCH):
                sl = slice(chi * CH, (chi + 1) * CH)
                bt_ps = ps.tile([DI, CH], f32, tag="mm")
                nc.tensor.matmul(bt_ps[:], W_B, XC[:, sl],
                                 start=True, stop=True)
                nc.scalar.activation(BT[:, sl], bt_ps[:], Act.Copy)
                ct_ps = ps.tile([DI, CH], f32, tag="mm")
                nc.tensor.matmul(ct_ps[:], W_C, XC[:, sl],
                                 start=True, stop=True)
                nc.vector.tensor_scalar(CT[:, sl], ct_ps[:], 0.0, None,
                                        op0=Alu.add)

            # ---- XCD = XC*Dp on Pool (runs during scan phase) ----
            XCD = bpool.tile([DI, L], bf16, name="XCD", tag="ESB")
            for hf in range(2):
                hsl = slice(hf * (L // 2), (hf + 1) * (L // 2))
                nc.gpsimd.tensor_tensor(XCD[:, hsl], XC[:, hsl],
                                        Dp.to_broadcast((DI, L // 2)),
                                        op=Alu.mult)

            # dummy exp: forces the exp act-table load to happen now (ACT
            # idle) instead of being chained onto the first dA exp's waits
            dumex = cpool.tile([128, 1], f32, tag="dumex")
            nc.scalar.activation(dumex[:], BT[:, 0:1], Act.Exp)

            # ---- scan phase: per channel-group g ----
            LH0 = L // 2
            dAe = bpool.tile([DI, LH0], f32, name="dAe", tag="XC")
            ypsA = [psy.tile([DI, CH], f32, name=f"ypsA{ci}", tag="yps")
                    for ci in range(4)]
            for g in range(NG):
                udt = udpool.tile([DI, 2 * L], bf16, tag="udt")
                for j in range(NS):
                    nc.sync.dma_start(
                        udt[j * 16:(j + 1) * 16, :],
                        UD[g * 16:(g + 1) * 16, :])
                for hf in range(2):
                    hsl = slice(hf * LH0, (hf + 1) * LH0)
                    # dbx in-place into the U-half of udt (dead after this)
                    nc.vector.tensor_tensor(udt[:, hsl], udt[:, hsl],
                                            BT[:, hsl], op=Alu.mult)
                    if hf == 0:
                        dA = dAe
                    else:
                        dA = dapool.tile([DI, LH0], f32, tag="dA")
                    nc.scalar.activation(
                        dA[:], udt[:, L + hf * LH0:L + (hf + 1) * LH0],
                        Act.Exp, scale=a_vec[:, g:g + 1])
                    init = 0.0 if hf == 0 else YPs[g][:, LH0 - 1:LH0]
                    nc.vector.tensor_tensor_scan(YPs[g][:, hsl], dA[:],
                                                 udt[:, hsl], init,
                                                 op0=Alu.mult, op1=Alu.add)
                    if g >= NG - 2 and hf == 0:
                        # last two groups' h0 y-mul into udt's dead dbx
                        # region (their pool buffers are never recycled):
                        # overlaps the h1 scan, drain starts earlier
                        if g == NG - 2:
                            nc.gpsimd.tensor_tensor(udt[:, hsl],
                                                    YPs[g][:, hsl],
                                                    CT[:, hsl], op=Alu.mult)
                        else:
                            nc.vector.tensor_tensor(udt[:, hsl],
                                                    YPs[g][:, hsl],
                                                    CT[:, hsl], op=Alu.mult)
                # y partial: YP = H * C_tile (even groups on Pool), split in
                # halves so h0 overlaps the h1 scan and accumulates earlier
                for hf in range(2):
                    hsl2 = slice(hf * LH0, (hf + 1) * LH0)
                    if g >= NG - 2 and hf == 0:
                        continue          # already done into udt above
                    if g != 5 and g != 7:
                        nc.gpsimd.tensor_tensor(YPs[g][:, hsl2], YPs[g][:, hsl2],
                                                CT[:, hsl2], op=Alu.mult)
                    else:
                        nc.vector.tensor_tensor(YPs[g][:, hsl2], YPs[g][:, hsl2],
                                                CT[:, hsl2], op=Alu.mult)
                # incremental y-reduce for chunks 0-3 (PSUM live across phase)
                for ci in range(4):
                    slc = slice(ci * CH, (ci + 1) * CH)
                    rhs = udt[:, slc] if g >= NG - 2 else YPs[g][:, slc]
                    nc.tensor.matmul(ypsA[ci][:], Rg[:, g * 128:(g + 1) * 128],
                                     rhs, start=(g == 0), stop=(g == NG - 1))

            # ---- y reduce: chunks 0-3 done incrementally; drain + chunks 4-7
            for chi in range(4):
                ysb = spool.tile([DI, CH], bf16, tag="ysb")
                nc.scalar.activation(ysb[:], ypsA[chi][:], Act.Copy)
                nc.sync.dma_start(
                    y_in_t[chi // 2, :, (chi % 2) * CH:(chi % 2 + 1) * CH],
                    ysb[:])
            for chi in range(4, NCH):
                sl = slice(chi * CH, (chi + 1) * CH)
                yps = psy.tile([DI, CH], f32, tag="yps")
                for g in range(NG):
                    nc.tensor.matmul(yps[:], Rg[:, g * 128:(g + 1) * 128],
                                     YPs[g][:, sl],
                                     start=(g == 0), stop=(g == NG - 1))
                ysb = spool.tile([DI, CH], bf16, tag="ysb")
                if chi % 2 == 0:
                    nc.scalar.activation(ysb[:], yps[:], Act.Copy)
                else:
                    nc.vector.tensor_scalar(ysb[:], yps[:], 0.0, None,
                                            op0=Alu.add)
                nc.sync.dma_start(
                    y_in_t[chi // 2, :, (chi % 2) * CH:(chi % 2 + 1) * CH],
                    ysb[:])

            # ---- AllReduce partial y in quarters, pipelined with post/out ----
            # XCD precomputed on Pool (overlaps scan phase)
            YS = bpool.tile([DI, L], bf16, name="YS", tag="HN")
            LQ = L // 4
            for q in range(4):
                qsl = slice(q * LQ, (q + 1) * LQ)
                if sim:
                    nc.sync.dma_start(y_out_t[q], y_in_t[q])
                else:
                    nc.gpsimd.collective_compute(
                        "AllReduce", Alu.add, replica_groups=groups,
                        ins=[y_in_t[q].opt()], outs=[y_out_t[q].opt()])
                nc.sync.dma_start(YSUM[:, qsl], y_out_t[q])
                nc.vector.tensor_add(XCD[:, qsl], YSUM[:, qsl], XCD[:, qsl])
                nc.vector.tensor_mul(YS[:, qsl], XCD[:, qsl], ZS[:, qsl])
                for ci in range(2):
                    chi = q * 2 + ci
                    sl = slice(chi * CH, (chi + 1) * CH)
                    op_ps = psy.tile([C, CH], f32, tag="yps")
                    nc.tensor.matmul(op_ps[:], op_lhsT, YS[:, sl],
                                     start=True, stop=True)
                    outc = spool.tile([C, CH], f32, tag="outc")
                    nc.vector.tensor_tensor(outc[:], op_ps[:], SEQ[:, sl],
                                            op=Alu.add)
                    nc.sync.dma_start(out_d[:, sl], outc[:])

    nc.compile()
    return nc


def _host_precompute(inp):
    import ml_dtypes
    f = lambda k: np.asarray(inp[k], np.float32)
    bf = lambda a: np.ascontiguousarray(a.astype(ml_dtypes.bfloat16))
    w1 = f("conv_w")[:, :, 0, 0]
    wh = f("dwh_w")[:, 0, :, 0]
    ww = f("dww_w")[:, 0, 0, :]
    s_bn = f("bn_g") / np.sqrt(f("bn_v") + EPS)
    taps = [
        w1 * (1.0 + wh[:, 1] + ww[:, 1])[None, :],   # center
        w1 * wh[:, 0][None, :],                       # up
        w1 * wh[:, 2][None, :],                       # down
        w1 * ww[:, 0][None, :],                       # left
        w1 * ww[:, 2][None, :],                       # right
    ]
    cw = np.concatenate([t.T for t in taps], axis=1)
    cw = cw * np.tile(s_bn, 5)[None, :]  # [cin=64, 5*64]
    btot = f("conv_b") + w1 @ (f("dwh_b") + f("dww_b"))
    bn_bias = s_bn * (btot - f("bn_m")) + f("bn_b")
    ipw = f("in_proj_w")
    ip_lhsT = (ipw * f("ln_g")[None, :]).T            # [64, 256]
    ip_bias = ipw @ f("ln_b")                          # [256]
    xpw = f("x_proj_w")                                # [36, 128]
    M_dt = f("dt_proj_w") @ xpw[:DR]                   # [128, 128]
    a_full = -np.exp(np.asarray(inp["A_log"], np.float32))  # [DI, DS]
    cdw = f("convd_w")[:, 0, :]                        # [128, 4]

    per_sigma = []
    for sg in range(2):
        s_lo = sg * NS
        cf32 = np.zeros((128, 32), np.float32)
        cf32[:C, 0] = s_bn
        cf32[:C, 1] = bn_bias
        cf32[:, 2] = ip_bias[:DI]
        cf32[:, 3] = ip_bias[DI:]
        cf32[:, 4] = f("convd_b")
        cf32[:, 5] = f("dt_proj_b")
        cf32[:, 6] = f("Dp")
        # a_vec per group g: a[p] = a_full[16g + p%16, s_lo + p//16]
        p = np.arange(128)
        for g in range(NG):
            cf32[:, 8 + g] = a_full[16 * g + p % 16, s_lo + p // 16]

        cbf = np.zeros((128, 2688), np.float32)
        cbf[:, 0:128] = np.eye(128, dtype=np.float32)
        cbf[:C, 128:448] = cw
        cbf[:C, 448:704] = ip_lhsT
        cbf[:, 704:832] = M_dt.T
        for tap in range(4):
            cbf[:, 832 + tap * 128:832 + (tap + 1) * 128] = np.diag(cdw[:, tap])
        # fused B/C broadcast: W_B[p, :] = xpw_B[s_lo + p//16, :] (stored T)
        for pp in range(128):
            cbf[:, 1344 + pp] = xpw[DR + s_lo + pp // 16]
            cbf[:, 1472 + pp] = xpw[DR + DS + s_lo + pp // 16]
        # Rg: R_g[p, d] = 1 iff d == 16g + p%16
        for g in range(NG):
            for pp in range(128):
                cbf[pp, 1600 + g * 128 + 16 * g + pp % 16] = 1.0
        cbf[:, 2624:2688] = f("out_proj_w").T
        per_sigma.append(dict(cf32=cf32, cbf=bf(cbf)))
    return {}, per_sigma


def _shift_images(xb):
    # 5 pre-shifted copies: ctr, up(reads h-1), dn(h+1), lf(w-1), rt(w+1)
    import ml_dtypes
    out = np.zeros((C, 5, H, W), np.float32)
    out[:, 0] = xb
    out[:, 1, 1:, :] = xb[:, :-1, :]
    out[:, 2, :-1, :] = xb[:, 1:, :]
    out[:, 3, :, 1:] = xb[:, :, :-1]
    out[:, 4, :, :-1] = xb[:, :, 1:]
    return np.ascontiguousarray(
        out.transpose(1, 0, 2, 3).reshape(5, C, L).transpose(1, 0, 2)
        .reshape(C, 5 * L).astype(ml_dtypes.bfloat16))


TRACE = False
LAST_EXEC_NS = None
LAST_TRACE_DIR = None


def kernel(**inputs):
    global LAST_EXEC_NS, LAST_TRACE_DIR
    from concourse.bass_utils import run_bass_kernel_spmd

    if "nc" not in _cached:
        _cached["nc"] = _build_program()
    nc = _cached["nc"]

    common, per_sigma = _host_precompute(inputs)
    x = np.asarray(inputs["x"], np.float32)
    in_maps = []
    for c in range(NCORES):
        b, sg = c // 2, c % 2
        m = dict(common)
        m.update(per_sigma[sg])
        m["ximgs"] = _shift_images(x[b])
        in_maps.append(m)

    kw = {}
    if TRACE:
        import tempfile
        LAST_TRACE_DIR = tempfile.mkdtemp(prefix="bass_trace_")
        kw = dict(trace=True, tmpdir=LAST_TRACE_DIR)
    r = run_bass_kernel_spmd(nc, in_maps, list(range(NCORES)), **kw)
    if r.exec_time_ns is not None:
        LAST_EXEC_NS = r.exec_time_ns
    res = r.results
    out = np.empty((B, C, H, W), np.float32)
    for b in range(B):
        out[b] = np.asarray(res[2 * b]["out_f"], np.float32).reshape(C, H, W)
    return out
